# revision 1
# baseline (speedup 1.0000x reference)
"""Multi-head attention (nn_Attention1D) on 8 Trainium2 NeuronCores.

Full inputs in, full output out.  Sharding: batch (2) x head-groups (4 heads
per core).  Each core computes, for its batch b and its 4 heads:
  q = (x_q @ WqT + bq)/sqrt(dk)   (scale folded into weights host-side)
  k = x_k @ WkT + bk
  v = x_v @ WvT + bv
  scoresT[sk,sq] = kT.T-style matmul (keeps softmax axis on partitions)
  p_un = exp(scoresT) * maskT          (exp(s + log(m)) == exp(s)*m)
  xattT_un[dk,sq] = v_ext.T @ p_un     (v_ext has a ones column -> row 64
                                        of the PSUM tile is the softmax
                                        denominator, for free)
  xattT = xattT_un * (1/denom)         (ACT exp(-ln) + K=1 broadcast matmul)
  outT_partial = WoT.T @ xattT         (partial over this core's 256 e-cols)
Host sums the 4 partial outT per batch and adds bo.
"""

import contextlib

import numpy as np

import concourse.bass as bass
import concourse.mybir as mybir
import concourse.tile as tile

F32 = mybir.dt.float32
BF16 = mybir.dt.bfloat16

# ---------------------------------------------------------------- config
import os

F16_P = os.environ.get("ATTN_F16_P", "1") == "1"  # exp/mask/p/v in bf16
F16_X = os.environ.get("ATTN_F16_X", "1") == "1"  # x inputs + qkv weights in bf16

P = 128
NB = 512  # psum bank in fp32 elements == max matmul N


def _split_multiwait(nc, max_waits=1):
    """This walrus build only accepts one sync wait per instruction; hoist
    extra waits onto NoOps inserted just before."""
    for bb in nc.main_func.blocks:
        new_insts = []
        for ins in bb.instructions:
            if ins.sync_info and len(ins.sync_info.on_wait) > max_waits:
                waits = list(ins.sync_info.on_wait)
                ins.sync_info.on_wait = waits[:max_waits]
                for i, w in enumerate(waits[max_waits:]):
                    nop = mybir.InstNoOp(name=f"{ins.name}_ws{i}", ins=[], outs=[])
                    nop.engine = ins.engine
                    nop.sync_info = mybir.SyncInfo(on_wait=[w], on_update=[])
                    nc.register_instruction(nop)
                    new_insts.append(nop)
            new_insts.append(ins)
        bb.instructions = new_insts

def build_program(D=1024, S=2048, E=256, DK=64, CS=512, CQ=1024, f16_p=F16_P, f16_x=F16_X):
    """Per-core attention program.

    The first head of the first head-pair runs its attention sweep
    interleaved with the projections (each k/v chunk feeds its sk-tiles
    immediately), so the Activation engine's exp stream - the co-critical
    resource - starts ~35us earlier.  PSUM budget during the overlap:
    projections rotate through 2x 1-bank slots, scores through 2x 2-bank
    slots, plus one 2-bank PV accumulator = 8 banks exactly.
    """
    assert f16_p and f16_x, "builder assumes bf16 p/mask/v and bf16 x inputs"
    H = E // DK
    DK1 = DK + 1
    KD = D // P
    KE = E // P
    SK = S // P
    CQ = min(CQ, S)
    NCS = S // CS
    NCQ = S // CQ
    SKC = CS // P  # sk-tiles produced per projection chunk
    QC0 = min(NCS, max(1, CQ // CS))  # q chunks needed before the first score
    F32R = mybir.dt.float32r
    XDT = BF16
    PDT = BF16
    EXP = mybir.ActivationFunctionType.Exp
    IDN = mybir.ActivationFunctionType.Identity
    LN = mybir.ActivationFunctionType.Ln

    nc = bass.Bass()
    xqT = nc.dram_tensor("xqT", [D, S], XDT, kind="ExternalInput")
    xkT = nc.dram_tensor("xkT", [D, S], XDT, kind="ExternalInput")
    xvT = nc.dram_tensor("xvT", [D, S], XDT, kind="ExternalInput")
    maskT = nc.dram_tensor("maskT", [S, S], PDT, kind="ExternalInput")
    wqT = nc.dram_tensor("wqT", [D, E], XDT, kind="ExternalInput")
    wkT = nc.dram_tensor("wkT", [D, E], XDT, kind="ExternalInput")
    wvT = nc.dram_tensor("wvT", [D, E], XDT, kind="ExternalInput")
    woT = nc.dram_tensor("woT", [E, D], F32R, kind="ExternalInput")
    bqT = nc.dram_tensor("bqT", [E, 1], F32, kind="ExternalInput")
    bkT = nc.dram_tensor("bkT", [E, 1], F32, kind="ExternalInput")
    bv = nc.dram_tensor("bv", [1, E], XDT, kind="ExternalInput")
    ones_r = nc.dram_tensor("ones_r", [P, 64], F32R, kind="ExternalInput")
    ones_x = nc.dram_tensor("ones_x", [1, CS], XDT, kind="ExternalInput")
    ones_p = nc.dram_tensor("ones_p", [1, P], PDT, kind="ExternalInput")
    outT = nc.dram_tensor("outT", [D, S], F32, kind="ExternalOutput")

    xqT_r = xqT.rearrange("(k p) s -> p k s", p=P)
    xkT_r = xkT.rearrange("(k p) s -> p k s", p=P)
    xvT_r = xvT.rearrange("(k p) s -> p k s", p=P)
    maskT_r = maskT.rearrange("(k p) s -> p k s", p=P)
    wqT_r = wqT.rearrange("(k p) e -> p k e", p=P)
    wkT_r = wkT.rearrange("(k p) e -> p k e", p=P)
    wvT_r = wvT.rearrange("(k p) e -> p k e", p=P)
    woT_r = woT.rearrange("(k p) d -> p k d", p=P)
    outT_r = outT.rearrange("(m p) s -> p m s", p=P)

    with tile.TileContext(nc) as tc:
        with (
            tc.tile_pool(name="persist", bufs=1) as persist,
            tc.tile_pool(name="consts", bufs=1) as consts,
            tc.tile_pool(name="bm", bufs=1) as bm,
            tc.tile_pool(name="dw", bufs=1) as dw,
            tc.tile_pool(name="be", bufs=4) as be,
            tc.tile_pool(name="bp", bufs=5) as bp,
            tc.tile_pool(name="bs", bufs=2) as bsc,
            tc.tile_pool(name="psX", bufs=1, space="PSUM") as psX,
        ):
            qT_sb = persist.tile([P, KE, S], F32R)
            kT_sb = persist.tile([P, KE, S], F32R)
            v_sb = persist.tile([P, SK, H, DK1], PDT)
            xattT_sb = persist.tile([P, KE, S], F32R)
            ones_sb = consts.tile([P, 64], F32R)
            onesx_sb = consts.tile([1, CS], XDT)
            bqT_sb = consts.tile([P, KE], F32)
            bkT_sb = consts.tile([P, KE], F32)
            bv_sb = consts.tile([1, E], XDT)
            nc.gpsimd.memset(v_sb[:, :, :, DK:DK1], 1.0)

            wo_sb = dw.tile([P, KE, D], F32R)
            mask0_sb = bm.tile([P, SK, CQ], PDT)
            masks = {0: mask0_sb}

            # ------------- emission helpers -------------
            def emit_q(cs):
                ssl = slice(cs * CS, (cs + 1) * CS)
                xq_sb = ax.tile([P, KD, CS], XDT, tag="xq")
                if cs == 0:
                    nc.sync.dma_start(
                        out=xq_sb[:, 0 : KD // 2, :], in_=xqT_r[:, 0 : KD // 2, ssl]
                    )
                    nc.sync.dma_start(
                        out=xq_sb[:, KD // 2 :, :], in_=xqT_r[:, KD // 2 :, ssl]
                    )
                    nc.sync.dma_start(
                        out=bqT_sb[:], in_=bqT.rearrange("(t p) o -> p (t o)", p=P)
                    )
                    nc.sync.dma_start(
                        out=bkT_sb[:], in_=bkT.rearrange("(t p) o -> p (t o)", p=P)
                    )
                    nc.sync.dma_start(out=bv_sb[:], in_=bv[:])
                    nc.sync.dma_start(out=onesx_sb[:], in_=ones_x[:])
                    nc.sync.dma_start(out=ones_sb[:], in_=ones_r[:])
                else:
                    nc.sync.dma_start(out=xq_sb[:], in_=xqT_r[:, :, ssl])
                for t in range(KE):
                    esl = slice(t * P, (t + 1) * P)
                    psq = aps.tile([P, CS], F32, tag="a")
                    for k in range(KD):
                        nc.tensor.matmul(
                            psq[:], wq_sb[:, k, esl], xq_sb[:, k, :],
                            start=(k == 0), stop=(k == KD - 1),
                        )
                    nc.vector.tensor_scalar_add(
                        qT_sb[:, t, ssl], psq[:], bqT_sb[:, t : t + 1]
                    )

            def emit_k(cs):
                ssl = slice(cs * CS, (cs + 1) * CS)
                xk_sb = ax.tile([P, KD, CS], XDT, tag="xk")
                if cs == 0:
                    nc.sync.dma_start(out=wk_sb[:], in_=wkT_r[:])
                nc.sync.dma_start(out=xk_sb[:], in_=xkT_r[:, :, ssl])
                for t in range(KE):
                    esl = slice(t * P, (t + 1) * P)
                    psk = aps.tile([P, CS], F32, tag="a")
                    for k in range(KD):
                        nc.tensor.matmul(
                            psk[:], wk_sb[:, k, esl], xk_sb[:, k, :],
                            start=(k == 0), stop=(k == KD - 1),
                        )
                    nc.vector.tensor_scalar_add(
                        kT_sb[:, t, ssl], psk[:], bkT_sb[:, t : t + 1]
                    )

            def emit_v(cs):
                ssl = slice(cs * CS, (cs + 1) * CS)
                xv_sb = ax.tile([P, KD, CS], XDT, tag="xv")
                if cs == 0:
                    nc.sync.dma_start(out=wv_sb[:], in_=wvT_r[:])
                nc.sync.dma_start(out=xv_sb[:], in_=xvT_r[:, :, ssl])
                # stream the first mask in per-chunk slices so its 4 MB
                # doesn't stall the x/weight loads in the SP queue; slice cs
                # covers exactly the sk-tiles this chunk's attention needs
                nc.sync.dma_start(
                    out=mask0_sb[:, cs * SKC : (cs + 1) * SKC, :],
                    in_=maskT_r[:, cs * SKC : (cs + 1) * SKC, 0:CQ],
                )
                for st in range(SKC):
                    stg = cs * SKC + st
                    psv = aps.tile([P, E], F32, tag="a")
                    for k in range(KD):
                        nc.tensor.matmul(
                            psv[:],
                            xv_sb[:, k, st * P : (st + 1) * P],
                            wv_sb[:, k, :],
                            start=(k == 0), stop=False,
                        )
                    nc.tensor.matmul(
                        psv[:], onesx_sb[:, :P], bv_sb[:],
                        start=False, stop=True,
                    )
                    nc.vector.tensor_copy(
                        v_sb[:, stg, :, 0:DK],
                        psv[:].rearrange("p (h d) -> p h d", h=H),
                    )

            def emit_pv(cq, hp, sk, xa, h2, pT):
                h = 2 * hp + h2
                for n in range(CQ // NB):
                    nc.tensor.matmul(
                        xa[h2][:, n * NB : (n + 1) * NB],
                        v_sb[:, sk, h, :],
                        pT[:, n * NB : (n + 1) * NB],
                        start=(sk == 0), stop=(sk == SK - 1),
                    )

            def emit_attn(cq, hp, sk, xa, heads=(0, 1), sp=None, pv=True):
                q0 = cq * CQ
                mask_sb = masks[cq]
                ps_s = {}
                for h2 in heads:
                    ps_s[h2] = sp[0].tile([P, CQ], F32, tag=sp[1], name=f"pss{h2}")
                    psl = slice(64 * h2, 64 * (h2 + 1))
                    for n in range(CQ // NB):
                        nc.tensor.matmul(
                            ps_s[h2][:, n * NB : (n + 1) * NB],
                            kT_sb[psl, hp, sk * P : (sk + 1) * P],
                            qT_sb[psl, hp, q0 + n * NB : q0 + (n + 1) * NB],
                            start=True, stop=True,
                        )
                stash = []
                for h2 in heads:
                    exp_sb = be.tile([P, CQ], PDT, tag="exp")
                    nc.scalar.activation(exp_sb[:], ps_s[h2][:], EXP)
                    pT = bp.tile([P, CQ], PDT, tag="pT")
                    nc.vector.tensor_mul(pT[:], exp_sb[:], mask_sb[:, sk, :])
                    if pv:
                        emit_pv(cq, hp, sk, xa, h2, pT)
                    else:
                        stash.append((h2, pT))
                return stash

            def emit_norm(cq, hp, xa, heads=(0, 1), sp=None):
                q0 = cq * CQ
                for h2 in heads:
                    rc1 = bsc.tile([P, CQ], F32, tag="rc1")
                    rc2 = bsc.tile([P, CQ], F32R, tag="rc2")
                    nc.scalar.activation(rc1[64:65, :], xa[h2][64:65, :], LN)
                    nc.scalar.activation(
                        rc2[64:65, :], rc1[64:65, :], EXP, scale=-1.0
                    )
                    bc_ps = sp[0].tile([64, CQ], F32, tag=sp[1])
                    for n in range(CQ // NB):
                        nc.tensor.matmul(
                            bc_ps[:, n * NB : (n + 1) * NB],
                            ones_sb[64:65, :],
                            rc2[64:65, n * NB : (n + 1) * NB],
                            start=True, stop=True,
                        )
                    bc_sb = bsc.tile([64, CQ], F32, tag="bc")
                    nc.vector.tensor_copy(bc_sb[:], bc_ps[:])
                    nc.vector.tensor_mul(
                        xattT_sb[64 * h2 : 64 * (h2 + 1), hp, q0 : q0 + CQ],
                        xa[h2][0:DK, :],
                        bc_sb[:],
                    )

            def emit_D(cq, sp=None):
                q0 = cq * CQ
                for m in range(D // P):
                    msl = slice(m * P, (m + 1) * P)
                    o_sb = do.tile([P, CQ], F32, tag="osb")
                    for n in range(CQ // NB):
                        nsl_l = slice(n * NB, (n + 1) * NB)
                        nsl_g = slice(q0 + n * NB, q0 + (n + 1) * NB)
                        ps_o = sp[0].tile([P, NB], F32, tag=sp[1])
                        for kk in range(KE):
                            nc.tensor.matmul(
                                ps_o[:],
                                wo_sb[:, kk, msl],
                                xattT_sb[:, kk, nsl_g],
                                start=(kk == 0), stop=(kk == KE - 1),
                            )
                        if (m + n) % 2 == 0:
                            nc.vector.tensor_copy(o_sb[:, nsl_l], ps_o[:])
                        else:
                            nc.scalar.copy(out=o_sb[:, nsl_l], in_=ps_o[:])
                    nc.gpsimd.dma_start(
                        out=outT_r[:, m, q0 : q0 + CQ], in_=o_sb[:]
                    )

            # ------------- projections + head 0 of pair 0 interleaved -----
            xa_h0 = psX.tile([DK1, CQ], F32, tag="xa0")
            with (
                tc.tile_pool(name="aw", bufs=1) as aw,
                tc.tile_pool(name="ax", bufs=2) as ax,
                tc.tile_pool(name="aps", bufs=2, space="PSUM") as aps,
            ):
                wq_sb = aw.tile([P, KD, E], XDT)
                wk_sb = aw.tile([P, KD, E], XDT)
                wv_sb = aw.tile([P, KD, E], XDT)
                nc.sync.dma_start(
                    out=wq_sb[:, 0 : KD // 2, :], in_=wqT_r[:, 0 : KD // 2, :]
                )
                nc.sync.dma_start(
                    out=wq_sb[:, KD // 2 :, :], in_=wqT_r[:, KD // 2 :, :]
                )

                for cs in range(QC0):
                    emit_q(cs)
                for cs in range(NCS):
                    emit_k(cs)
                    emit_v(cs)
                    for sk in range(cs * SKC, (cs + 1) * SKC):
                        emit_attn(0, 0, sk, {0: xa_h0}, heads=(0,), sp=(aps, "sA"))
                for cs in range(QC0, NCS):
                    emit_q(cs)

            # ------------- rest of the attention + output projection -----
            with (
                tc.tile_pool(name="bm2", bufs=1) as bm2,
                tc.tile_pool(name="do", bufs=4) as do,
                tc.tile_pool(name="psS", bufs=2, space="PSUM") as psS,
                tc.tile_pool(name="psX2", bufs=1, space="PSUM") as psX2,
            ):
                nc.sync.dma_start(out=wo_sb[:], in_=woT_r[:])
                for cq in range(1, NCQ):
                    m_sb = bm2.tile([P, SK, CQ], PDT, tag="mask2")
                    nc.sync.dma_start(
                        out=m_sb[:], in_=maskT_r[:, :, cq * CQ : (cq + 1) * CQ]
                    )
                    masks[cq] = m_sb

                # head 1 of pair 0 (solo; head 0 already accumulated)
                xa_h1 = psX2.tile([DK1, CQ], F32, tag="xa1")
                for sk in range(SK):
                    emit_attn(0, 0, sk, {1: xa_h1}, heads=(1,), sp=(psS, "s"))

                # Pipeline pair boundaries with a SINGLE-SLOT warmup: emit only
                # head 0 of the next pair's sk=0 before the previous pair's
                # normalize, so PE/ACT have filler during the recip chain while
                # the second PSUM score slot stays free for the broadcast.
                pending = [lambda: emit_norm(0, 0, {0: xa_h0, 1: xa_h1}, sp=(psS, "s"))]

                def start_pair(cq, hp, then_D=None):
                    xn0 = psX.tile([DK1, CQ], F32, tag="xa0", name="xn0")
                    xn1 = psX2.tile([DK1, CQ], F32, tag="xa1", name="xn1")
                    xn = [xn0, xn1]
                    stash0 = emit_attn(
                        cq, hp, 0, xn, heads=(0,), sp=(psS, "s"), pv=False
                    )
                    stash1 = emit_attn(
                        cq, hp, 1, xn, heads=(0,), sp=(psS, "s"), pv=False
                    )
                    for fn in pending:
                        fn()
                    pending.clear()
                    for h2, pT in stash0:
                        emit_pv(cq, hp, 0, xn, h2, pT)
                    emit_attn(cq, hp, 0, xn, heads=(1,), sp=(psS, "s"))
                    for h2, pT in stash1:
                        emit_pv(cq, hp, 1, xn, h2, pT)
                    emit_attn(cq, hp, 1, xn, heads=(1,), sp=(psS, "s"))
                    for sk in range(2, SK):
                        emit_attn(cq, hp, sk, xn, sp=(psS, "s"))
                    pending.append(lambda: emit_norm(cq, hp, xn, sp=(psS, "s")))
                    if then_D is not None:
                        pending.append(lambda: emit_D(then_D, sp=(psS, "s")))

                pairs = [(0, hp) for hp in range(1, H // 2)]
                for cq in range(1, NCQ):
                    pairs += [(cq, hp) for hp in range(H // 2)]
                for cq, hp in pairs:
                    start_pair(cq, hp, then_D=cq - 1 if hp == 0 and cq >= 1 else None)
                for fn in pending:
                    fn()
                emit_D(NCQ - 1, sp=(psS, "s"))

    _split_multiwait(nc, 1)
    return nc


# ---------------------------------------------------------------- host side

B, S_FULL, D_FULL, H_FULL = 2, 2048, 1024, 16
DK_FULL = D_FULL // H_FULL
N_CORES = 8
GROUPS = N_CORES // B  # head-groups per batch
EG = D_FULL // GROUPS  # e-columns per core

_NC_CACHE = {}


def _get_program():
    key = "full"
    if key not in _NC_CACHE:
        _NC_CACHE[key] = build_program(D=D_FULL, S=S_FULL, E=EG, DK=DK_FULL)
    return _NC_CACHE[key]


def _cast(a, f16):
    a = np.ascontiguousarray(a, dtype=np.float32)
    if f16:
        import ml_dtypes

        return a.astype(ml_dtypes.bfloat16)
    return a


LAST_RES = None


def kernel(query, key, value, softmask, Wq, bq, Wk, bk, Wv, bv, Wo, bo, _trace=False):
    global LAST_RES
    from concourse.bass_utils import run_bass_kernel_spmd

    nc = _get_program()
    scale = 1.0 / np.sqrt(np.float32(DK_FULL))

    in_maps = []
    for c in range(N_CORES):
        b, g = c // GROUPS, c % GROUPS
        es = slice(g * EG, (g + 1) * EG)
        m = {
            "xqT": _cast(query[b].T, F16_X),
            "xkT": _cast(key[b].T, F16_X),
            "xvT": _cast(value[b].T, F16_X),
            "maskT": _cast(softmask[b].T + 1e-30, F16_P),
            "wqT": _cast(Wq[es, :].T * scale, F16_X),
            "wkT": _cast(Wk[es, :].T, F16_X),
            "wvT": _cast(Wv[es, :].T, F16_X),
            "woT": _cast(Wo[:, es].T, False),
            "bqT": _cast(bq[es, None] * scale, False),
            "bkT": _cast(bk[es, None], False),
            "bv": _cast(bv[None, es], F16_X),
            "ones_r": np.ones((128, 64), np.float32),
            "ones_x": _cast(np.ones((1, 512)), F16_X),
            "ones_p": _cast(np.ones((1, 128)), F16_P),
        }
        in_maps.append(m)

    res = run_bass_kernel_spmd(
        nc, in_maps, core_ids=list(range(N_CORES)), trace=_trace
    )
    LAST_RES = res

    out = np.zeros((B, S_FULL, D_FULL), dtype=np.float32)
    for c in range(N_CORES):
        b = c // GROUPS
        out[b] += res.results[c]["outT"].T
    out += np.asarray(bo, dtype=np.float32)[None, None, :]
    return out



# revision 18
# speedup vs baseline: 1.1885x; 1.1885x over previous
"""Multi-head attention (nn_Attention1D) on 8 Trainium2 NeuronCores.

Full inputs in, full output out.  Sharding: batch (2) x head-groups (4 heads
per core, E=256 e-columns).  Per-core pipeline (ACT exp stream is the
critical resource; everything else hides under it):

  QKV projections: compensated fp8 DoubleRow matmuls (3 terms:
      xh@wh + (xl*4)@(wh/4) + (xh/4)@(wl*4), weights pre-scaled by 64 into
      e4m3's normal range, rescaled in the bias-add) -> bf16-level accuracy
      at 1/4 the PE cost of bf16.  q/k stored bf16 [dk, s]; v stored
      bf16 [s, (h, dk|1)] with a ones column (softmax denominator for free).
  scores:   scoresT[sk, q] = kT.T @ qT per (head, sk-tile), fp32 PSUM.
  softmax:  ACT exp -> bf16; DVE multiply by softmask tile -> pT (bf16).
  PV:       flipped orientation: stationary = pT tile [k,q], moving =
            v [k, 65] -> xa[q, 64|denom] accumulated over sk (2x fewer
            streamed columns than the [dk, q] orientation).
  norm:     DVE reciprocal of the denom column + per-partition scalar mul.
  out-proj: PE-transpose xatt [q,e] -> xattT [e,q] via identity matmuls,
            then out[q, d] = xattT.T @ wo, copies on GPSIMD, bf16 out.
  Host sums the 4 per-core partials per batch and adds bo.
"""

import math
from collections import deque

import numpy as np

import concourse.bass as bass
import concourse.mybir as mybir
import concourse.tile as tile

F32 = mybir.dt.float32
BF16 = mybir.dt.bfloat16
F8 = mybir.dt.float8e4
DR = mybir.MatmulPerfMode.DoubleRow
EXP = mybir.ActivationFunctionType.Exp
MULT = mybir.AluOpType.mult
ADD = mybir.AluOpType.add

P = 128
WS = 64.0  # weight pre-scale into e4m3 normal range


def _split_multiwait(nc, max_waits=1):
    """This walrus build only accepts one sync wait per instruction; hoist
    extra waits onto NoOps inserted just before."""
    for bb in nc.main_func.blocks:
        new_insts = []
        for ins in bb.instructions:
            if ins.sync_info and len(ins.sync_info.on_wait) > max_waits:
                waits = list(ins.sync_info.on_wait)
                ins.sync_info.on_wait = waits[:max_waits]
                for i, w in enumerate(waits[max_waits:]):
                    nop = mybir.InstNoOp(name=f"{ins.name}_ws{i}", ins=[], outs=[])
                    nop.engine = ins.engine
                    nop.sync_info = mybir.SyncInfo(on_wait=[w], on_update=[])
                    nc.register_instruction(nop)
                    new_insts.append(nop)
            new_insts.append(ins)
        bb.instructions = new_insts


def build_program(D=1024, S=2048, E=256, DK=64):
    H = E // DK          # 4 heads per core
    KE = E // P          # 2 e-tiles
    KT = D // 256        # 4 DoubleRow k-tiles (K=256 each)
    SK = S // P          # 16 sk-tiles
    CS = 512             # projection chunk (s columns)
    NCS = S // CS        # 4
    CQ = 1024            # attention q chunk
    NCQ = S // CQ        # 2
    QS = CQ // P         # 8 q-subtiles per chunk
    DK1 = DK + 1

    nc = bass.Bass()
    xq8 = nc.dram_tensor("xq8", [P, S // 512, 2, KT, 2, 512], F8, kind="ExternalInput")
    xk8 = nc.dram_tensor("xk8", [P, S // 512, 2, KT, 2, 512], F8, kind="ExternalInput")
    xv8 = nc.dram_tensor("xv8", [P, S // 512, 2, KT, 2, 512], F8, kind="ExternalInput")
    wq8 = nc.dram_tensor("wq8", [P, 3, KT, 2, E], F8, kind="ExternalInput")
    wk8 = nc.dram_tensor("wk8", [P, 3, KT, 2, E], F8, kind="ExternalInput")
    wv8 = nc.dram_tensor("wv8", [P, 3, KT, 2, E], F8, kind="ExternalInput")
    wo = nc.dram_tensor("wo", [P, KE, D], BF16, kind="ExternalInput")
    bqT = nc.dram_tensor("bqT", [P, KE], F32, kind="ExternalInput")
    bkT = nc.dram_tensor("bkT", [P, KE], F32, kind="ExternalInput")
    bvw = nc.dram_tensor("bvw", [1, E], BF16, kind="ExternalInput")
    ones_c = nc.dram_tensor("ones_c", [1, P], BF16, kind="ExternalInput")
    ident = nc.dram_tensor("ident", [P, P], BF16, kind="ExternalInput")
    maskT = nc.dram_tensor("maskT", [P, SK, S], BF16, kind="ExternalInput")
    out = nc.dram_tensor("out", [P, S // P, D], BF16, kind="ExternalOutput")

    with tile.TileContext(nc) as tc:
        with (
            tc.tile_pool(name="persist", bufs=1) as persist,
            tc.tile_pool(name="ax", bufs=4) as ax,
            tc.tile_pool(name="bm", bufs=16) as bm,
            tc.tile_pool(name="be", bufs=5) as be,
            tc.tile_pool(name="bp", bufs=2) as bp,
            tc.tile_pool(name="bxa", bufs=2) as bxa,
            tc.tile_pool(name="bxt", bufs=1) as bxt,
            tc.tile_pool(name="bo", bufs=2) as bo_,
            tc.tile_pool(name="brc", bufs=4) as brc,
            tc.tile_pool(name="psS", bufs=2, space="PSUM") as psS,
            tc.tile_pool(name="psV", bufs=2, space="PSUM") as psV,
            tc.tile_pool(name="psO", bufs=2, space="PSUM") as psO,
        ):
            qT_sb = persist.tile([P, KE, S], BF16)
            kT_sb = persist.tile([P, KE, S], BF16)
            v_sb = persist.tile([P, SK, H, DK1], BF16)
            wq_sb = persist.tile([P, 3, KT, 2, E], F8)
            wk_sb = persist.tile([P, 3, KT, 2, E], F8)
            wv_sb = persist.tile([P, 3, KT, 2, E], F8)
            wo_sb = persist.tile([P, KE, D], BF16)
            bq_sb = persist.tile([P, KE], F32)
            bk_sb = persist.tile([P, KE], F32)
            bvw_sb = persist.tile([1, E], BF16)
            ones_sb = persist.tile([1, P], BF16)
            id_sb = persist.tile([P, P], BF16)
            nc.gpsimd.memset(v_sb[:, :, :, DK:DK1], 1.0)

            TERMS = [(0, 0), (1, 1), (0, 2)]  # (x ver, w ver): xh@wh + xl4@wh4 + xh@wl

            # ---------------- emission helpers ----------------
            x_tiles = {}

            def issue_x(which, c):
                xd = {"q": xq8, "k": xk8, "v": xv8}[which]
                xt = ax.tile([P, 2, KT, 2, CS], F8, tag="x", name=f"x{which}{c}")
                nc.gpsimd.dma_start(out=xt[:], in_=xd[:, c])
                x_tiles[(which, c)] = xt

            def emit_q(c, which):
                w_sb, b_sb, t_sb = {
                    "q": (wq_sb, bq_sb, qT_sb),
                    "k": (wk_sb, bk_sb, kT_sb),
                }[which]
                ssl = slice(c * CS, (c + 1) * CS)
                xt = x_tiles.pop((which, c))
                for et in range(KE):
                    esl = slice(et * P, (et + 1) * P)
                    ps = psV.tile([P, CS], F32, tag="v")
                    n = 0
                    for xv, wv in TERMS:
                        for kt in range(KT):
                            nc.tensor.matmul(
                                ps[:], w_sb[:, wv, kt, :, esl], xt[:, xv, kt, :, :],
                                start=(n == 0), stop=(n == 3 * KT - 1),
                                perf_mode=DR,
                            )
                            n += 1
                    nc.vector.tensor_scalar(
                        out=t_sb[:, et, ssl], in0=ps[:],
                        scalar1=1.0 / WS, scalar2=b_sb[:, et : et + 1],
                        op0=MULT, op1=ADD,
                    )

            xv_tiles = {}

            def emit_v(c, st):
                xt = x_tiles[("v", c)]
                stg = c * (CS // P) + st
                psl = slice(st * P, (st + 1) * P)
                ps = psO.tile([P, E], F32, tag="o2")
                n = 0
                for xv, wv in TERMS:
                    for kt in range(KT):
                        nc.tensor.matmul(
                            ps[:], xt[:, xv, kt, :, psl], wv_sb[:, wv, kt, :, :],
                            start=(n == 0), stop=False, perf_mode=DR,
                        )
                        n += 1
                nc.tensor.matmul(ps[:], ones_sb[:], bvw_sb[:], start=False, stop=True)
                nc.vector.tensor_scalar(
                    out=v_sb[:, stg, :, 0:DK],
                    in0=ps[:].rearrange("p (h d) -> p h d", h=H),
                    scalar1=1.0 / WS, scalar2=None, op0=MULT,
                )

            def mk_pv(pTh, h, qsub, xatt_t):
                def f():
                    xa = psV.tile([P, DK1], F32, tag="v")
                    qsl = slice(qsub * P, (qsub + 1) * P)
                    for sk in range(SK):
                        nc.tensor.matmul(
                            xa[:], pTh[:, sk, qsl], v_sb[:, sk, h, :],
                            start=(sk == 0), stop=(sk == SK - 1),
                        )
                    rec = brc.tile([P, 1], F32, tag="rc")
                    nc.vector.reciprocal(rec[:], xa[:, DK:DK1])
                    nc.vector.tensor_scalar(
                        out=xatt_t[:, qsub, h * DK : (h + 1) * DK],
                        in0=xa[:, 0:DK], scalar1=rec[:], scalar2=None, op0=MULT,
                    )
                    return SK * DK1 + 500
                return f

            def mk_tr(xatt_t, xaT_t, qsub, pool=None, ptag="o2", use_act=False):
                def f():
                    for et in range(KE):
                        pt = (pool or psO).tile([P, P], BF16, tag=ptag, name="pt")
                        nc.tensor.transpose(
                            pt[:], xatt_t[:, qsub, et * P : (et + 1) * P], id_sb[:]
                        )
                        dst = xaT_t[:, et, qsub * P : (qsub + 1) * P]
                        if use_act:
                            nc.scalar.copy(out=dst, in_=pt[:])
                        else:
                            nc.vector.tensor_copy(dst, pt[:])
                    return 2 * P + 400
                return f

            def mk_op(xaT_t, cq, qsub, use_act=False):
                def f():
                    ot = bo_.tile([P, D], BF16, tag="o")
                    qsl = slice(qsub * P, (qsub + 1) * P)
                    for dn in range(D // 512):
                        po = psO.tile([P, 512], F32, tag="o2")
                        dsl = slice(dn * 512, (dn + 1) * 512)
                        for et in range(KE):
                            nc.tensor.matmul(
                                po[:], xaT_t[:, et, qsl], wo_sb[:, et, dsl],
                                start=(et == 0), stop=(et == KE - 1),
                            )
                        if use_act and dn == 0:
                            nc.scalar.copy(out=ot[:, dsl], in_=po[:])
                        else:
                            nc.vector.tensor_copy(ot[:, dsl], po[:])
                    nc.gpsimd.dma_start(out=out[:, cq * QS + qsub, :], in_=ot[:])
                    return 2 * D + 600
                return f

            # ---------------- schedule ----------------
            pending = deque()  # (tag, cost_estimate, closure)

            def pull(budget):
                while pending and budget > 0:
                    tag, cost, f = pending.popleft()
                    r = f()
                    budget -= cost if r is None else r

            def flush(tag_needed):
                while any(t == tag_needed for t, _, _ in pending):
                    t, cost, f = pending.popleft()
                    f()

            # PE warmup: ramp the p-state to full clock before the first
            # projection data lands (dummy matmuls on a zeroed tile)
            warm_sb = persist.tile([1, 512], BF16, name="warm_sb")
            nc.gpsimd.memset(warm_sb[:], 0.0)
            for wi in range(12):
                ps_w = psS.tile([P, 512], F32, tag="s", name="wps")
                nc.tensor.matmul(
                    ps_w[:], warm_sb[:, 0:P], warm_sb[:], start=True, stop=True
                )

            # head: minimal DMA chain to the first scores: wq,xq0 / wk,xk0 / xq1
            # (head x chunks on the SP queue — they carry no WAR waits; later
            # chunks go through the Pool queue whose waits don't block issue)
            def issue_x_sp(which, c):
                xd = {"q": xq8, "k": xk8, "v": xv8}[which]
                xt = ax.tile([P, 2, KT, 2, CS], F8, tag="x", name=f"x{which}{c}")
                nc.sync.dma_start(out=xt[:], in_=xd[:, c])
                x_tiles[(which, c)] = xt

            nc.sync.dma_start(out=wq_sb[:], in_=wq8[:])
            nc.sync.dma_start(out=bq_sb[:], in_=bqT[:])
            issue_x_sp("q", 0)
            nc.sync.dma_start(out=wk_sb[:], in_=wk8[:])
            nc.sync.dma_start(out=bk_sb[:], in_=bkT[:])
            issue_x_sp("k", 0)
            issue_x_sp("q", 1)
            issue_x_sp("k", 1)
            emit_q(0, "q")
            emit_q(0, "k")
            emit_q(1, "q")

            mask_tiles = {}

            def mask_dma(cq, sk):
                mt = bm.tile([P, CQ], BF16, tag="m", name=f"m{cq}_{sk}")
                nc.sync.dma_start(
                    out=mt[:], in_=maskT[:, sk, cq * CQ : (cq + 1) * CQ]
                )
                mask_tiles[sk] = mt

            xatt_tiles = {}
            xaT_tiles = {}

            for cq in range(NCQ):
                xatt_t = bxa.tile([P, QS, E], BF16, tag="xatt")
                xatt_tiles[cq] = xatt_t
                mask_dma(cq, 0)
                mask_dma(cq, 1)
                for h in range(H):
                    half, ke = h & 1, h >> 1
                    pdsl = slice(64 * half, 64 * half + 64)
                    pTh = bp.tile([P, SK, CQ], BF16, tag="pT", name=f"pT{cq}_{h}")
                    for sk in range(SK):
                        if cq == 0 and h == 0:
                            if sk % 4 == 0 and sk > 0:
                                emit_q(sk // 4, "k")
                            if sk == 1:
                                nc.sync.dma_start(out=wv_sb[:], in_=wv8[:])
                                nc.sync.dma_start(out=bvw_sb[:], in_=bvw[:])
                                nc.sync.dma_start(out=ones_sb[:], in_=ones_c[:])
                                issue_x("k", 2)
                            elif sk == 2:
                                issue_x("k", 3)
                            elif sk in (5, 7, 9):
                                issue_x("v", (sk - 5) // 2)
                            elif sk == 13:
                                issue_x("v", 3)
                        if cq == 0 and h == 1 and sk == 0:
                            nc.sync.dma_start(out=id_sb[:], in_=ident[:])
                            nc.sync.dma_start(out=wo_sb[:], in_=wo[:])
                        if cq == 0 and h == 2 and sk in (0, 2):
                            issue_x("q", 2 + sk // 2)
                        if h == 0 and sk + 2 < SK:
                            mask_dma(cq, sk + 2)
                        ss = psS.tile([P, CQ], F32, tag="s")
                        halves = 2 if (cq == 0 and h == 0 and sk < 4) else 1
                        et_t = be.tile([P, CQ], BF16, tag="e")
                        for piece in range(halves):
                            psl_ = slice(piece * (CQ // halves), (piece + 1) * (CQ // halves))
                            for n2 in range(CQ // 512 // halves):
                                base = piece * (CQ // halves) + n2 * 512
                                nsl = slice(base, base + 512)
                                gsl = slice(cq * CQ + base, cq * CQ + base + 512)
                                nc.tensor.matmul(
                                    ss[:, nsl], kT_sb[pdsl, ke, sk * P : (sk + 1) * P],
                                    qT_sb[pdsl, ke, gsl], start=True, stop=True,
                                )
                            nc.scalar.activation(et_t[:, psl_], ss[:, psl_], EXP)
                            nc.vector.tensor_mul(
                                pTh[:, sk, psl_], et_t[:, psl_], mask_tiles[sk][:, psl_]
                            )
                        if cq == 0 and h == 0:
                            pull(200)
                        else:
                            pull(2400 if sk < 6 else 1200)
                    # post-head work
                    if cq == 0 and h == 0:
                        for c in range(NCS):
                            for st in range(CS // P):
                                pending.append(
                                    ("v", 1900, (lambda c=c, st=st: emit_v(c, st)))
                                )
                    if cq == 0 and h == 2:
                        pending.append(("proj", 3400, lambda: emit_q(2, "q")))
                        pending.append(("proj", 3400, lambda: emit_q(3, "q")))
                    if h < H - 1:
                        for qsub in range(QS):
                            pending.append(("pv", SK * DK1 + 500, mk_pv(pTh, h, qsub, xatt_t)))
                    else:
                        # stagger PV with transpose/out-proj so the per-qsub
                        # chains pipeline through the 2-slot psum pools
                        xaT_t = bxt.tile([P, KE, CQ], BF16, tag="xaT")
                        xaT_tiles[cq] = xaT_t
                        last = cq == NCQ - 1
                        tr_pool, tr_tag = (psS, "s") if last else (None, "o2")
                        for qsub in range(QS):
                            pending.append(("pv", SK * DK1 + 500, mk_pv(pTh, h, qsub, xatt_t)))
                            if qsub >= 1:
                                j = qsub - 1
                                pending.append(("tr", 2 * P + 400, mk_tr(xatt_t, xaT_t, j, tr_pool, tr_tag, last)))
                                pending.append(("op", 2 * D + 600, mk_op(xaT_t, cq, j, last)))
                        for j in (QS - 1,):
                            pending.append(("tr", 2 * P + 400, mk_tr(xatt_t, xaT_t, j, tr_pool, tr_tag, last)))
                            pending.append(("op", 2 * D + 600, mk_op(xaT_t, cq, j, last)))
                if cq == 0:
                    flush("proj")
            while pending:
                _, _, f = pending.popleft()
                f()

    _split_multiwait(nc, 1)
    return nc


# ---------------------------------------------------------------- host side

B, S_FULL, D_FULL, H_FULL = 2, 2048, 1024, 16
DK_FULL = D_FULL // H_FULL
N_CORES = 8
GROUPS = N_CORES // B   # head-groups per batch
EG = D_FULL // GROUPS   # e-columns per core

_NC_CACHE = {}


def _get_program():
    if "full" not in _NC_CACHE:
        _NC_CACHE["full"] = build_program(D=D_FULL, S=S_FULL, E=EG, DK=DK_FULL)
    return _NC_CACHE["full"]


def _f8(a):
    import ml_dtypes

    return a.astype(ml_dtypes.float8_e4m3fn)


def _bf(a):
    import ml_dtypes

    return np.ascontiguousarray(a, dtype=np.float32).astype(ml_dtypes.bfloat16)


def _prep_x(aT, ncols):
    """[1024, ncols] f32 -> [128, ncols//512, 2, 4, 2, 512] fp8 (hi, lo*4)
    with d = 256*kt + 128*u + p; chunk-major for big-descriptor DMA."""
    a = np.ascontiguousarray(aT, dtype=np.float32)
    hi = _f8(a)
    lo4 = _f8((a - hi.astype(np.float32)) * 4.0)
    v = np.stack([hi, lo4], axis=0)
    v = v.reshape(2, 4, 2, 128, ncols).transpose(3, 0, 1, 2, 4)
    v = v.reshape(128, 2, 4, 2, ncols // 512, 512).transpose(0, 4, 1, 2, 3, 5)
    return np.ascontiguousarray(v)


def _prep_w(aT, ncols):
    """[1024, ncols] f32 (pre-scaled by WS) -> [128, 3, 4, 2, ncols] fp8
    versions (wh, wh/4, wl)."""
    a = np.ascontiguousarray(aT, dtype=np.float32)
    wh = _f8(a)
    whf = wh.astype(np.float32)
    wh4 = _f8(whf / 4.0)
    wl = _f8(a - whf)
    v = np.stack([wh, wh4, wl], axis=0)
    v = v.reshape(3, 4, 2, 128, ncols).transpose(3, 0, 1, 2, 4)
    return np.ascontiguousarray(v)


LAST_RES = None


def kernel(query, key, value, softmask, Wq, bq, Wk, bk, Wv, bv, Wo, bo, _trace=False):
    global LAST_RES
    from concourse.bass_utils import run_bass_kernel_spmd

    nc = _get_program()
    scale = np.float32(1.0 / math.sqrt(DK_FULL))

    x_cache = {}
    for b in range(B):
        x_cache[b] = (
            _prep_x(np.asarray(query[b], np.float32).T, S_FULL),
            _prep_x(np.asarray(key[b], np.float32).T, S_FULL),
            _prep_x(np.asarray(value[b], np.float32).T, S_FULL),
            np.ascontiguousarray(
                _bf(np.asarray(softmask[b], np.float32).T + 1e-30)
                .reshape(S_FULL // 128, 128, S_FULL)
                .transpose(1, 0, 2)
            ),
        )

    ident = _bf(np.eye(128, dtype=np.float32))
    ones_c = _bf(np.ones((1, 128), np.float32))

    in_maps = []
    for c in range(N_CORES):
        b, g = c // GROUPS, c % GROUPS
        es = slice(g * EG, (g + 1) * EG)
        xq8, xk8, xv8, mT = x_cache[b]
        m = {
            "xq8": xq8, "xk8": xk8, "xv8": xv8, "maskT": mT,
            "wq8": _prep_w(Wq[es, :].T * (scale * WS), EG),
            "wk8": _prep_w(Wk[es, :].T * WS, EG),
            "wv8": _prep_w(Wv[es, :].T * WS, EG),
            "wo": np.ascontiguousarray(
                _bf(Wo[:, es].T).reshape(EG // 128, 128, D_FULL).transpose(1, 0, 2)
            ),
            "bqT": np.ascontiguousarray(
                (np.asarray(bq[es], np.float32) * scale).reshape(EG // 128, 128).T
            ),
            "bkT": np.ascontiguousarray(
                np.asarray(bk[es], np.float32).reshape(EG // 128, 128).T
            ),
            "bvw": _bf(np.asarray(bv[es], np.float32)[None, :] * WS),
            "ones_c": ones_c,
            "ident": ident,
        }
        in_maps.append(m)

    res = run_bass_kernel_spmd(
        nc, in_maps, core_ids=list(range(N_CORES)), trace=_trace
    )
    LAST_RES = res

    outp = np.zeros((B, S_FULL, D_FULL), dtype=np.float32)
    for c in range(N_CORES):
        b = c // GROUPS
        o = res.results[c]["out"].astype(np.float32)  # [128, 16, D]
        outp[b] += o.transpose(1, 0, 2).reshape(S_FULL, D_FULL)
    outp += np.asarray(bo, dtype=np.float32)[None, None, :]
    return outp


# revision 25
# speedup vs baseline: 1.2082x; 1.0166x over previous
"""Multi-head attention (nn_Attention1D) on 8 Trainium2 NeuronCores.

Full inputs in, full output out.  Sharding: batch (2) x head-groups (4 heads
per core, E=256 e-columns).  Per-core pipeline (ACT exp stream is the
critical resource; everything else hides under it):

  QKV projections: compensated fp8 DoubleRow matmuls (3 terms:
      xh@wh + (xl*4)@(wh/4) + (xh/4)@(wl*4), weights pre-scaled by 64 into
      e4m3's normal range, rescaled in the bias-add) -> bf16-level accuracy
      at 1/4 the PE cost of bf16.  q/k stored bf16 [dk, s]; v stored
      bf16 [s, (h, dk|1)] with a ones column (softmax denominator for free).
  scores:   scoresT[sk, q] = kT.T @ qT per (head, sk-tile), fp32 PSUM.
  softmax:  ACT exp -> bf16; DVE multiply by softmask tile -> pT (bf16).
  PV:       flipped orientation: stationary = pT tile [k,q], moving =
            v [k, 65] -> xa[q, 64|denom] accumulated over sk (2x fewer
            streamed columns than the [dk, q] orientation).
  norm:     DVE reciprocal of the denom column + per-partition scalar mul.
  out-proj: PE-transpose xatt [q,e] -> xattT [e,q] via identity matmuls,
            then out[q, d] = xattT.T @ wo, copies on GPSIMD, bf16 out.
  Host sums the 4 per-core partials per batch and adds bo.
"""

import math
from collections import deque

import numpy as np

import concourse.bass as bass
import concourse.mybir as mybir
import concourse.tile as tile

F32 = mybir.dt.float32
BF16 = mybir.dt.bfloat16
F8 = mybir.dt.float8e4
DR = mybir.MatmulPerfMode.DoubleRow
EXP = mybir.ActivationFunctionType.Exp
MULT = mybir.AluOpType.mult
ADD = mybir.AluOpType.add

P = 128
WS = 64.0  # weight pre-scale into e4m3 normal range


def _split_multiwait(nc, max_waits=1):
    """This walrus build only accepts one sync wait per instruction; hoist
    extra waits onto NoOps inserted just before."""
    for bb in nc.main_func.blocks:
        new_insts = []
        for ins in bb.instructions:
            if ins.sync_info and len(ins.sync_info.on_wait) > max_waits:
                waits = list(ins.sync_info.on_wait)
                ins.sync_info.on_wait = waits[:max_waits]
                for i, w in enumerate(waits[max_waits:]):
                    nop = mybir.InstNoOp(name=f"{ins.name}_ws{i}", ins=[], outs=[])
                    nop.engine = ins.engine
                    nop.sync_info = mybir.SyncInfo(on_wait=[w], on_update=[])
                    nc.register_instruction(nop)
                    new_insts.append(nop)
            new_insts.append(ins)
        bb.instructions = new_insts


def build_program(D=1024, S=2048, E=256, DK=64):
    H = E // DK          # 4 heads per core
    KE = E // P          # 2 e-tiles
    KT = D // 256        # 4 DoubleRow k-tiles (K=256 each)
    SK = S // P          # 16 sk-tiles
    CS = 512             # projection chunk (s columns)
    NCS = S // CS        # 4
    CQ = 1024            # attention q chunk
    NCQ = S // CQ        # 2
    QS = CQ // P         # 8 q-subtiles per chunk
    DK1 = DK + 1

    nc = bass.Bass()
    xq8 = nc.dram_tensor("xq8", [P, S // 512, 2, KT, 2, 512], F8, kind="ExternalInput")
    xk8 = nc.dram_tensor("xk8", [P, S // 512, 2, KT, 2, 512], F8, kind="ExternalInput")
    xv8 = nc.dram_tensor("xv8", [P, S // 512, 2, KT, 2, 512], F8, kind="ExternalInput")
    wq8 = nc.dram_tensor("wq8", [P, 3, KT, 2, E], F8, kind="ExternalInput")
    wk8 = nc.dram_tensor("wk8", [P, 3, KT, 2, E], F8, kind="ExternalInput")
    wv8 = nc.dram_tensor("wv8", [P, 3, KT, 2, E], F8, kind="ExternalInput")
    wo = nc.dram_tensor("wo", [P, KE, D], BF16, kind="ExternalInput")
    bqT = nc.dram_tensor("bqT", [P, KE], F32, kind="ExternalInput")
    bkT = nc.dram_tensor("bkT", [P, KE], F32, kind="ExternalInput")
    bvw = nc.dram_tensor("bvw", [1, E], BF16, kind="ExternalInput")
    ones_c = nc.dram_tensor("ones_c", [1, P], BF16, kind="ExternalInput")
    ident = nc.dram_tensor("ident", [P, P], BF16, kind="ExternalInput")
    maskT = nc.dram_tensor("maskT", [P, SK, S], BF16, kind="ExternalInput")
    out = nc.dram_tensor("out", [P, S // P, D], BF16, kind="ExternalOutput")

    with tile.TileContext(nc) as tc:
        with (
            tc.tile_pool(name="persist", bufs=1) as persist,
            tc.tile_pool(name="ax", bufs=4) as ax,
            tc.tile_pool(name="bm", bufs=16) as bm,
            tc.tile_pool(name="be", bufs=5) as be,
            tc.tile_pool(name="bp", bufs=2) as bp,
            tc.tile_pool(name="bxa", bufs=2) as bxa,
            tc.tile_pool(name="bxt", bufs=1) as bxt,
            tc.tile_pool(name="bo", bufs=2) as bo_,
            tc.tile_pool(name="brc", bufs=4) as brc,
            tc.tile_pool(name="psS", bufs=2, space="PSUM") as psS,
            tc.tile_pool(name="psV", bufs=2, space="PSUM") as psV,
            tc.tile_pool(name="psO", bufs=2, space="PSUM") as psO,
        ):
            qT_sb = persist.tile([P, KE, S], BF16)
            kT_sb = persist.tile([P, KE, S], BF16)
            v_sb = persist.tile([P, SK, H, DK1], BF16)
            wq_sb = persist.tile([P, 3, KT, 2, E], F8)
            wk_sb = persist.tile([P, 3, KT, 2, E], F8)
            wv_sb = persist.tile([P, 3, KT, 2, E], F8)
            wo_sb = persist.tile([P, KE, D], BF16)
            bq_sb = persist.tile([P, KE], F32)
            bk_sb = persist.tile([P, KE], F32)
            bvw_sb = persist.tile([1, E], BF16)
            ones_sb = persist.tile([1, P], BF16)
            id_sb = persist.tile([P, P], BF16)
            nc.gpsimd.memset(v_sb[:, :, :, DK:DK1], 1.0)

            TERMS = [(0, 0), (1, 1), (0, 2)]  # (x ver, w ver): xh@wh + xl4@wh4 + xh@wl

            # ---------------- emission helpers ----------------
            x_tiles = {}

            def issue_x(which, c):
                xd = {"q": xq8, "k": xk8, "v": xv8}[which]
                xt = ax.tile([P, 2, KT, 2, CS], F8, tag="x", name=f"x{which}{c}")
                nc.gpsimd.dma_start(out=xt[:], in_=xd[:, c])
                x_tiles[(which, c)] = xt

            def emit_q(c, which):
                w_sb, b_sb, t_sb = {
                    "q": (wq_sb, bq_sb, qT_sb),
                    "k": (wk_sb, bk_sb, kT_sb),
                }[which]
                ssl = slice(c * CS, (c + 1) * CS)
                xt = x_tiles.pop((which, c))
                for et in range(KE):
                    esl = slice(et * P, (et + 1) * P)
                    ps = psV.tile([P, CS], F32, tag="v")
                    n = 0
                    for xv, wv in TERMS:
                        for kt in range(KT):
                            nc.tensor.matmul(
                                ps[:], w_sb[:, wv, kt, :, esl], xt[:, xv, kt, :, :],
                                start=(n == 0), stop=(n == 3 * KT - 1),
                                perf_mode=DR,
                            )
                            n += 1
                    nc.vector.tensor_scalar(
                        out=t_sb[:, et, ssl], in0=ps[:],
                        scalar1=1.0 / WS, scalar2=b_sb[:, et : et + 1],
                        op0=MULT, op1=ADD,
                    )

            xv_tiles = {}

            def emit_v(c, st):
                xt = x_tiles[("v", c)]
                stg = c * (CS // P) + st
                psl = slice(st * P, (st + 1) * P)
                ps = psO.tile([P, E], F32, tag="o2")
                n = 0
                for xv, wv in TERMS:
                    for kt in range(KT):
                        nc.tensor.matmul(
                            ps[:], xt[:, xv, kt, :, psl], wv_sb[:, wv, kt, :, :],
                            start=(n == 0), stop=False, perf_mode=DR,
                        )
                        n += 1
                nc.tensor.matmul(ps[:], ones_sb[:], bvw_sb[:], start=False, stop=True)
                nc.vector.tensor_scalar(
                    out=v_sb[:, stg, :, 0:DK],
                    in0=ps[:].rearrange("p (h d) -> p h d", h=H),
                    scalar1=1.0 / WS, scalar2=None, op0=MULT,
                )

            def mk_pv(pTh, h, qsub, xatt_t):
                def f():
                    xa = psV.tile([P, DK1], F32, tag="v")
                    qsl = slice(qsub * P, (qsub + 1) * P)
                    for sk in range(SK):
                        nc.tensor.matmul(
                            xa[:], pTh[:, sk, qsl], v_sb[:, sk, h, :],
                            start=(sk == 0), stop=(sk == SK - 1),
                        )
                    rec = brc.tile([P, 1], F32, tag="rc")
                    nc.vector.reciprocal(rec[:], xa[:, DK:DK1])
                    nc.vector.tensor_scalar(
                        out=xatt_t[:, qsub, h * DK : (h + 1) * DK],
                        in0=xa[:, 0:DK], scalar1=rec[:], scalar2=None, op0=MULT,
                    )
                    return SK * DK1 + 500
                return f

            def mk_tr(xatt_t, xaT_t, qsub, pool=None, ptag="o2", use_act=False):
                def f():
                    for et in range(KE):
                        pt = (pool or psO).tile([P, P], BF16, tag=ptag, name="pt")
                        nc.tensor.transpose(
                            pt[:], xatt_t[:, qsub, et * P : (et + 1) * P], id_sb[:]
                        )
                        dst = xaT_t[:, et, qsub * P : (qsub + 1) * P]
                        if use_act:
                            nc.scalar.copy(out=dst, in_=pt[:])
                        else:
                            nc.vector.tensor_copy(dst, pt[:])
                    return 2 * P + 400
                return f

            def mk_op(xaT_t, cq, qsub, use_act=False):
                def f():
                    ot = bo_.tile([P, D], BF16, tag="o")
                    qsl = slice(qsub * P, (qsub + 1) * P)
                    for dn in range(D // 512):
                        po = psO.tile([P, 512], F32, tag="o2")
                        dsl = slice(dn * 512, (dn + 1) * 512)
                        for et in range(KE):
                            nc.tensor.matmul(
                                po[:], xaT_t[:, et, qsl], wo_sb[:, et, dsl],
                                start=(et == 0), stop=(et == KE - 1),
                            )
                        if use_act and dn == 0:
                            nc.scalar.copy(out=ot[:, dsl], in_=po[:])
                        else:
                            nc.vector.tensor_copy(ot[:, dsl], po[:])
                    nc.gpsimd.dma_start(out=out[:, cq * QS + qsub, :], in_=ot[:])
                    return 2 * D + 600
                return f

            # ---------------- schedule ----------------
            pending = deque()  # (tag, cost_estimate, closure)

            def pull(budget):
                while pending and budget > 0:
                    tag, cost, f = pending.popleft()
                    r = f()
                    budget -= cost if r is None else r

            def flush(tag_needed):
                while any(t == tag_needed for t, _, _ in pending):
                    t, cost, f = pending.popleft()
                    f()

            # PE warmup: ramp the p-state to full clock before the first
            # projection data lands (dummy matmuls on a zeroed tile)
            warm_sb = persist.tile([1, 512], BF16, name="warm_sb")
            nc.gpsimd.memset(warm_sb[:], 0.0)
            for wi in range(12):
                ps_w = psS.tile([P, 512], F32, tag="s", name="wps")
                nc.tensor.matmul(
                    ps_w[:], warm_sb[:, 0:P], warm_sb[:], start=True, stop=True
                )

            # head: minimal DMA chain to the first scores: wq,xq0 / wk,xk0 / xq1
            # (head x chunks on the SP queue — they carry no WAR waits; later
            # chunks go through the Pool queue whose waits don't block issue)
            def issue_x_sp(which, c):
                xd = {"q": xq8, "k": xk8, "v": xv8}[which]
                xt = ax.tile([P, 2, KT, 2, CS], F8, tag="x", name=f"x{which}{c}")
                nc.sync.dma_start(out=xt[:], in_=xd[:, c])
                x_tiles[(which, c)] = xt

            nc.sync.dma_start(out=wq_sb[:], in_=wq8[:])
            nc.sync.dma_start(out=bq_sb[:], in_=bqT[:])
            issue_x_sp("q", 0)
            nc.sync.dma_start(out=wk_sb[:], in_=wk8[:])
            nc.sync.dma_start(out=bk_sb[:], in_=bkT[:])
            issue_x_sp("k", 0)
            issue_x_sp("q", 1)
            issue_x_sp("k", 1)
            emit_q(0, "q")
            emit_q(0, "k")
            emit_q(1, "q")
            nc.sync.dma_start(out=wv_sb[:], in_=wv8[:])
            nc.sync.dma_start(out=bvw_sb[:], in_=bvw[:])
            nc.sync.dma_start(out=ones_sb[:], in_=ones_c[:])

            mask_tiles = {}

            def mask_dma(cq, sk):
                mt = bm.tile([P, CQ], BF16, tag="m", name=f"m{cq}_{sk}")
                nc.sync.dma_start(
                    out=mt[:], in_=maskT[:, sk, cq * CQ : (cq + 1) * CQ]
                )
                mask_tiles[sk] = mt

            xatt_tiles = {}
            xaT_tiles = {}

            for cq in range(NCQ):
                xatt_t = bxa.tile([P, QS, E], BF16, tag="xatt")
                xatt_tiles[cq] = xatt_t
                mask_dma(cq, 0)
                mask_dma(cq, 1)
                for h in range(H):
                    half, ke = h & 1, h >> 1
                    pdsl = slice(64 * half, 64 * half + 64)
                    pTh = bp.tile([P, SK, CQ], BF16, tag="pT", name=f"pT{cq}_{h}")
                    for sk in range(SK):
                        if cq == 0 and h == 0:
                            if sk % 4 == 0 and sk > 0:
                                emit_q(sk // 4, "k")
                            if sk == 1:
                                issue_x("k", 2)
                            elif sk == 2:
                                issue_x("k", 3)
                            elif sk in (5, 7, 9):
                                issue_x("v", (sk - 5) // 2)
                            elif sk == 13:
                                issue_x("v", 3)
                        if cq == 0 and h == 1 and sk == 0:
                            nc.sync.dma_start(out=id_sb[:], in_=ident[:])
                            nc.sync.dma_start(out=wo_sb[:], in_=wo[:])
                        if cq == 0 and h == 2 and sk in (0, 2):
                            issue_x("q", 2 + sk // 2)
                        if h == 0 and sk + 2 < SK:
                            mask_dma(cq, sk + 2)
                        ss = psS.tile([P, CQ], F32, tag="s")
                        halves = 2 if (cq == 0 and h == 0 and sk < 4) else 1
                        et_t = be.tile([P, CQ], BF16, tag="e")
                        for piece in range(halves):
                            psl_ = slice(piece * (CQ // halves), (piece + 1) * (CQ // halves))
                            for n2 in range(CQ // 512 // halves):
                                base = piece * (CQ // halves) + n2 * 512
                                nsl = slice(base, base + 512)
                                gsl = slice(cq * CQ + base, cq * CQ + base + 512)
                                nc.tensor.matmul(
                                    ss[:, nsl], kT_sb[pdsl, ke, sk * P : (sk + 1) * P],
                                    qT_sb[pdsl, ke, gsl], start=True, stop=True,
                                )
                            nc.scalar.activation(et_t[:, psl_], ss[:, psl_], EXP)
                            nc.vector.tensor_mul(
                                pTh[:, sk, psl_], et_t[:, psl_], mask_tiles[sk][:, psl_]
                            )
                        if cq == 0 and h == 0:
                            pull(200)
                        else:
                            pull(2400 if sk < 6 else 1200)
                    # post-head work
                    if cq == 0 and h == 0:
                        for c in range(NCS):
                            for st in range(CS // P):
                                pending.append(
                                    ("v", 1900, (lambda c=c, st=st: emit_v(c, st)))
                                )
                    if cq == 0 and h == 2:
                        pending.append(("proj", 3400, lambda: emit_q(2, "q")))
                        pending.append(("proj", 3400, lambda: emit_q(3, "q")))
                    if h < H - 1:
                        for qsub in range(QS):
                            pending.append(("pv", SK * DK1 + 500, mk_pv(pTh, h, qsub, xatt_t)))
                    else:
                        # stagger PV with transpose/out-proj so the per-qsub
                        # chains pipeline through the 2-slot psum pools
                        xaT_t = bxt.tile([P, KE, CQ], BF16, tag="xaT")
                        xaT_tiles[cq] = xaT_t
                        last = cq == NCQ - 1
                        tr_pool, tr_tag = (psS, "s") if last else (None, "o2")
                        for qsub in range(QS):
                            pending.append(("pv", SK * DK1 + 500, mk_pv(pTh, h, qsub, xatt_t)))
                            if qsub >= 1:
                                j = qsub - 1
                                pending.append(("tr", 2 * P + 400, mk_tr(xatt_t, xaT_t, j, tr_pool, tr_tag, last)))
                                pending.append(("op", 2 * D + 600, mk_op(xaT_t, cq, j, last)))
                        for j in (QS - 1,):
                            pending.append(("tr", 2 * P + 400, mk_tr(xatt_t, xaT_t, j, tr_pool, tr_tag, last)))
                            pending.append(("op", 2 * D + 600, mk_op(xaT_t, cq, j, last)))
                if cq == 0:
                    flush("proj")
            while pending:
                _, _, f = pending.popleft()
                f()

    _split_multiwait(nc, 1)
    return nc


# ---------------------------------------------------------------- host side

B, S_FULL, D_FULL, H_FULL = 2, 2048, 1024, 16
DK_FULL = D_FULL // H_FULL
N_CORES = 8
GROUPS = N_CORES // B   # head-groups per batch
EG = D_FULL // GROUPS   # e-columns per core

_NC_CACHE = {}


def _get_program():
    if "full" not in _NC_CACHE:
        _NC_CACHE["full"] = build_program(D=D_FULL, S=S_FULL, E=EG, DK=DK_FULL)
    return _NC_CACHE["full"]


def _f8(a):
    import ml_dtypes

    return a.astype(ml_dtypes.float8_e4m3fn)


def _bf(a):
    import ml_dtypes

    return np.ascontiguousarray(a, dtype=np.float32).astype(ml_dtypes.bfloat16)


def _prep_x(aT, ncols):
    """[1024, ncols] f32 -> [128, ncols//512, 2, 4, 2, 512] fp8 (hi, lo*4)
    with d = 256*kt + 128*u + p; chunk-major for big-descriptor DMA."""
    a = np.ascontiguousarray(aT, dtype=np.float32)
    hi = _f8(a)
    lo4 = _f8((a - hi.astype(np.float32)) * 4.0)
    v = np.stack([hi, lo4], axis=0)
    v = v.reshape(2, 4, 2, 128, ncols).transpose(3, 0, 1, 2, 4)
    v = v.reshape(128, 2, 4, 2, ncols // 512, 512).transpose(0, 4, 1, 2, 3, 5)
    return np.ascontiguousarray(v)


def _prep_w(aT, ncols):
    """[1024, ncols] f32 (pre-scaled by WS) -> [128, 3, 4, 2, ncols] fp8
    versions (wh, wh/4, wl)."""
    a = np.ascontiguousarray(aT, dtype=np.float32)
    wh = _f8(a)
    whf = wh.astype(np.float32)
    wh4 = _f8(whf / 4.0)
    wl = _f8(a - whf)
    v = np.stack([wh, wh4, wl], axis=0)
    v = v.reshape(3, 4, 2, 128, ncols).transpose(3, 0, 1, 2, 4)
    return np.ascontiguousarray(v)


LAST_RES = None


def kernel(query, key, value, softmask, Wq, bq, Wk, bk, Wv, bv, Wo, bo, _trace=False):
    global LAST_RES
    from concourse.bass_utils import run_bass_kernel_spmd

    nc = _get_program()
    scale = np.float32(1.0 / math.sqrt(DK_FULL))

    x_cache = {}
    for b in range(B):
        x_cache[b] = (
            _prep_x(np.asarray(query[b], np.float32).T, S_FULL),
            _prep_x(np.asarray(key[b], np.float32).T, S_FULL),
            _prep_x(np.asarray(value[b], np.float32).T, S_FULL),
            np.ascontiguousarray(
                _bf(np.asarray(softmask[b], np.float32).T + 1e-30)
                .reshape(S_FULL // 128, 128, S_FULL)
                .transpose(1, 0, 2)
            ),
        )

    ident = _bf(np.eye(128, dtype=np.float32))
    ones_c = _bf(np.ones((1, 128), np.float32))

    in_maps = []
    for c in range(N_CORES):
        b, g = c // GROUPS, c % GROUPS
        es = slice(g * EG, (g + 1) * EG)
        xq8, xk8, xv8, mT = x_cache[b]
        m = {
            "xq8": xq8, "xk8": xk8, "xv8": xv8, "maskT": mT,
            "wq8": _prep_w(Wq[es, :].T * (scale * WS), EG),
            "wk8": _prep_w(Wk[es, :].T * WS, EG),
            "wv8": _prep_w(Wv[es, :].T * WS, EG),
            "wo": np.ascontiguousarray(
                _bf(Wo[:, es].T).reshape(EG // 128, 128, D_FULL).transpose(1, 0, 2)
            ),
            "bqT": np.ascontiguousarray(
                (np.asarray(bq[es], np.float32) * scale).reshape(EG // 128, 128).T
            ),
            "bkT": np.ascontiguousarray(
                np.asarray(bk[es], np.float32).reshape(EG // 128, 128).T
            ),
            "bvw": _bf(np.asarray(bv[es], np.float32)[None, :] * WS),
            "ones_c": ones_c,
            "ident": ident,
        }
        in_maps.append(m)

    res = run_bass_kernel_spmd(
        nc, in_maps, core_ids=list(range(N_CORES)), trace=_trace
    )
    LAST_RES = res

    outp = np.zeros((B, S_FULL, D_FULL), dtype=np.float32)
    for c in range(N_CORES):
        b = c // GROUPS
        o = res.results[c]["out"].astype(np.float32)  # [128, 16, D]
        outp[b] += o.transpose(1, 0, 2).reshape(S_FULL, D_FULL)
    outp += np.asarray(bo, dtype=np.float32)[None, None, :]
    return outp


# revision 28
# speedup vs baseline: 1.2135x; 1.0043x over previous
"""Multi-head attention (nn_Attention1D) on 8 Trainium2 NeuronCores.

Full inputs in, full output out.  Sharding: batch (2) x head-groups (4 heads
per core, E=256 e-columns).  Per-core pipeline (ACT exp stream is the
critical resource; everything else hides under it):

  QKV projections: compensated fp8 DoubleRow matmuls (3 terms:
      xh@wh + (xl*4)@(wh/4) + (xh/4)@(wl*4), weights pre-scaled by 64 into
      e4m3's normal range, rescaled in the bias-add) -> bf16-level accuracy
      at 1/4 the PE cost of bf16.  q/k stored bf16 [dk, s]; v stored
      bf16 [s, (h, dk|1)] with a ones column (softmax denominator for free).
  scores:   scoresT[sk, q] = kT.T @ qT per (head, sk-tile), fp32 PSUM.
  softmax:  ACT exp -> bf16; DVE multiply by softmask tile -> pT (bf16).
  PV:       flipped orientation: stationary = pT tile [k,q], moving =
            v [k, 65] -> xa[q, 64|denom] accumulated over sk (2x fewer
            streamed columns than the [dk, q] orientation).
  norm:     DVE reciprocal of the denom column + per-partition scalar mul.
  out-proj: PE-transpose xatt [q,e] -> xattT [e,q] via identity matmuls,
            then out[q, d] = xattT.T @ wo, copies on GPSIMD, bf16 out.
  Host sums the 4 per-core partials per batch and adds bo.
"""

import math
from collections import deque

import numpy as np

import concourse.bass as bass
import concourse.mybir as mybir
import concourse.tile as tile

F32 = mybir.dt.float32
BF16 = mybir.dt.bfloat16
F8 = mybir.dt.float8e4
DR = mybir.MatmulPerfMode.DoubleRow
EXP = mybir.ActivationFunctionType.Exp
MULT = mybir.AluOpType.mult
ADD = mybir.AluOpType.add

P = 128
WS = 64.0  # weight pre-scale into e4m3 normal range


def _split_multiwait(nc, max_waits=1):
    """This walrus build only accepts one sync wait per instruction; hoist
    extra waits onto NoOps inserted just before."""
    for bb in nc.main_func.blocks:
        new_insts = []
        for ins in bb.instructions:
            if ins.sync_info and len(ins.sync_info.on_wait) > max_waits:
                waits = list(ins.sync_info.on_wait)
                ins.sync_info.on_wait = waits[:max_waits]
                for i, w in enumerate(waits[max_waits:]):
                    nop = mybir.InstNoOp(name=f"{ins.name}_ws{i}", ins=[], outs=[])
                    nop.engine = ins.engine
                    nop.sync_info = mybir.SyncInfo(on_wait=[w], on_update=[])
                    nc.register_instruction(nop)
                    new_insts.append(nop)
            new_insts.append(ins)
        bb.instructions = new_insts


def build_program(D=1024, S=2048, E=256, DK=64):
    H = E // DK          # 4 heads per core
    KE = E // P          # 2 e-tiles
    KT = D // 256        # 4 DoubleRow k-tiles (K=256 each)
    SK = S // P          # 16 sk-tiles
    CS = 512             # projection chunk (s columns)
    NCS = S // CS        # 4
    CQ = 1024            # attention q chunk
    NCQ = S // CQ        # 2
    QS = CQ // P         # 8 q-subtiles per chunk
    DK1 = DK + 1

    nc = bass.Bass()
    xq8 = nc.dram_tensor("xq8", [P, S // 512, 2, KT, 2, 512], F8, kind="ExternalInput")
    xk8 = nc.dram_tensor("xk8", [P, S // 512, 2, KT, 2, 512], F8, kind="ExternalInput")
    xv8 = nc.dram_tensor("xv8", [P, S // 512, 2, KT, 2, 512], F8, kind="ExternalInput")
    wq8 = nc.dram_tensor("wq8", [P, 3, KT, 2, E], F8, kind="ExternalInput")
    wk8 = nc.dram_tensor("wk8", [P, 3, KT, 2, E], F8, kind="ExternalInput")
    wv8 = nc.dram_tensor("wv8", [P, 3, KT, 2, E], F8, kind="ExternalInput")
    wo = nc.dram_tensor("wo", [P, KE, D], BF16, kind="ExternalInput")
    bqT = nc.dram_tensor("bqT", [P, KE], F32, kind="ExternalInput")
    bkT = nc.dram_tensor("bkT", [P, KE], F32, kind="ExternalInput")
    bvw = nc.dram_tensor("bvw", [1, E], BF16, kind="ExternalInput")
    ones_c = nc.dram_tensor("ones_c", [1, P], BF16, kind="ExternalInput")
    ident = nc.dram_tensor("ident", [P, P], BF16, kind="ExternalInput")
    maskT = nc.dram_tensor("maskT", [P, SK, S], BF16, kind="ExternalInput")
    out = nc.dram_tensor("out", [P, S // P, D], BF16, kind="ExternalOutput")

    with tile.TileContext(nc) as tc:
        with (
            tc.tile_pool(name="persist", bufs=1) as persist,
            tc.tile_pool(name="ax", bufs=4) as ax,
            tc.tile_pool(name="bm", bufs=16) as bm,
            tc.tile_pool(name="be", bufs=6) as be,
            tc.tile_pool(name="bp", bufs=2) as bp,
            tc.tile_pool(name="bxa", bufs=2) as bxa,
            tc.tile_pool(name="bxt", bufs=1) as bxt,
            tc.tile_pool(name="bo", bufs=2) as bo_,
            tc.tile_pool(name="brc", bufs=4) as brc,
            tc.tile_pool(name="psS", bufs=2, space="PSUM") as psS,
            tc.tile_pool(name="psV", bufs=2, space="PSUM") as psV,
            tc.tile_pool(name="psO", bufs=2, space="PSUM") as psO,
        ):
            qT_sb = persist.tile([P, KE, S], BF16)
            kT_sb = persist.tile([P, KE, S], BF16)
            v_sb = persist.tile([P, SK, H, DK1], BF16)
            wq_sb = persist.tile([P, 3, KT, 2, E], F8)
            wk_sb = persist.tile([P, 3, KT, 2, E], F8)
            wv_sb = persist.tile([P, 3, KT, 2, E], F8)
            wo_sb = persist.tile([P, KE, D], BF16)
            bq_sb = persist.tile([P, KE], F32)
            bk_sb = persist.tile([P, KE], F32)
            bvw_sb = persist.tile([1, E], BF16)
            ones_sb = persist.tile([1, P], BF16)
            id_sb = persist.tile([P, P], BF16)
            nc.gpsimd.memset(v_sb[:, :, :, DK:DK1], 1.0)

            TERMS = [(0, 0), (1, 1), (0, 2)]  # (x ver, w ver): xh@wh + xl4@wh4 + xh@wl

            # ---------------- emission helpers ----------------
            x_tiles = {}

            def issue_x(which, c):
                xd = {"q": xq8, "k": xk8, "v": xv8}[which]
                xt = ax.tile([P, 2, KT, 2, CS], F8, tag="x", name=f"x{which}{c}")
                nc.gpsimd.dma_start(out=xt[:], in_=xd[:, c])
                x_tiles[(which, c)] = xt

            def emit_q(c, which):
                w_sb, b_sb, t_sb = {
                    "q": (wq_sb, bq_sb, qT_sb),
                    "k": (wk_sb, bk_sb, kT_sb),
                }[which]
                ssl = slice(c * CS, (c + 1) * CS)
                xt = x_tiles.pop((which, c))
                for et in range(KE):
                    esl = slice(et * P, (et + 1) * P)
                    ps = psV.tile([P, CS], F32, tag="v")
                    n = 0
                    for xv, wv in TERMS:
                        for kt in range(KT):
                            nc.tensor.matmul(
                                ps[:], w_sb[:, wv, kt, :, esl], xt[:, xv, kt, :, :],
                                start=(n == 0), stop=(n == 3 * KT - 1),
                                perf_mode=DR,
                            )
                            n += 1
                    nc.vector.tensor_scalar(
                        out=t_sb[:, et, ssl], in0=ps[:],
                        scalar1=1.0 / WS, scalar2=b_sb[:, et : et + 1],
                        op0=MULT, op1=ADD,
                    )

            xv_tiles = {}

            def emit_v(c, st):
                xt = x_tiles[("v", c)]
                stg = c * (CS // P) + st
                psl = slice(st * P, (st + 1) * P)
                ps = psO.tile([P, E], F32, tag="o2")
                n = 0
                for xv, wv in TERMS:
                    for kt in range(KT):
                        nc.tensor.matmul(
                            ps[:], xt[:, xv, kt, :, psl], wv_sb[:, wv, kt, :, :],
                            start=(n == 0), stop=False, perf_mode=DR,
                        )
                        n += 1
                nc.tensor.matmul(ps[:], ones_sb[:], bvw_sb[:], start=False, stop=True)
                nc.vector.tensor_scalar(
                    out=v_sb[:, stg, :, 0:DK],
                    in0=ps[:].rearrange("p (h d) -> p h d", h=H),
                    scalar1=1.0 / WS, scalar2=None, op0=MULT,
                )

            def mk_pv(pTh, h, qsub, xatt_t):
                def f():
                    xa = psV.tile([P, DK1], F32, tag="v")
                    qsl = slice(qsub * P, (qsub + 1) * P)
                    for sk in range(SK):
                        nc.tensor.matmul(
                            xa[:], pTh[:, sk, qsl], v_sb[:, sk, h, :],
                            start=(sk == 0), stop=(sk == SK - 1),
                        )
                    rec = brc.tile([P, 1], F32, tag="rc")
                    nc.vector.reciprocal(rec[:], xa[:, DK:DK1])
                    nc.vector.tensor_scalar(
                        out=xatt_t[:, qsub, h * DK : (h + 1) * DK],
                        in0=xa[:, 0:DK], scalar1=rec[:], scalar2=None, op0=MULT,
                    )
                    return SK * DK1 + 500
                return f

            def mk_tr(xatt_t, xaT_t, qsub, pool=None, ptag="o2", use_act=False):
                def f():
                    for et in range(KE):
                        pt = (pool or psO).tile([P, P], BF16, tag=ptag, name="pt")
                        nc.tensor.transpose(
                            pt[:], xatt_t[:, qsub, et * P : (et + 1) * P], id_sb[:]
                        )
                        dst = xaT_t[:, et, qsub * P : (qsub + 1) * P]
                        if use_act:
                            nc.scalar.copy(out=dst, in_=pt[:])
                        else:
                            nc.vector.tensor_copy(dst, pt[:])
                    return 2 * P + 400
                return f

            def mk_op(xaT_t, cq, qsub, use_act=False):
                def f():
                    ot = bo_.tile([P, D], BF16, tag="o")
                    qsl = slice(qsub * P, (qsub + 1) * P)
                    for dn in range(D // 512):
                        po = psO.tile([P, 512], F32, tag="o2")
                        dsl = slice(dn * 512, (dn + 1) * 512)
                        for et in range(KE):
                            nc.tensor.matmul(
                                po[:], xaT_t[:, et, qsl], wo_sb[:, et, dsl],
                                start=(et == 0), stop=(et == KE - 1),
                            )
                        if use_act and dn == 0:
                            nc.scalar.copy(out=ot[:, dsl], in_=po[:])
                        else:
                            nc.vector.tensor_copy(ot[:, dsl], po[:])
                    nc.gpsimd.dma_start(out=out[:, cq * QS + qsub, :], in_=ot[:])
                    return 2 * D + 600
                return f

            # ---------------- schedule ----------------
            pending = deque()  # (tag, cost_estimate, closure)

            def pull(budget):
                while pending and budget > 0:
                    tag, cost, f = pending.popleft()
                    r = f()
                    budget -= cost if r is None else r

            def flush(tag_needed):
                while any(t == tag_needed for t, _, _ in pending):
                    t, cost, f = pending.popleft()
                    f()

            # PE warmup: ramp the p-state to full clock before the first
            # projection data lands (dummy matmuls on a zeroed tile)
            warm_sb = persist.tile([1, 512], BF16, name="warm_sb")
            nc.gpsimd.memset(warm_sb[:], 0.0)
            for wi in range(12):
                ps_w = psS.tile([P, 512], F32, tag="s", name="wps")
                nc.tensor.matmul(
                    ps_w[:], warm_sb[:, 0:P], warm_sb[:], start=True, stop=True
                )

            # head: minimal DMA chain to the first scores: wq,xq0 / wk,xk0 / xq1
            # (head x chunks on the SP queue — they carry no WAR waits; later
            # chunks go through the Pool queue whose waits don't block issue)
            def issue_x_sp(which, c):
                xd = {"q": xq8, "k": xk8, "v": xv8}[which]
                xt = ax.tile([P, 2, KT, 2, CS], F8, tag="x", name=f"x{which}{c}")
                nc.sync.dma_start(out=xt[:], in_=xd[:, c])
                x_tiles[(which, c)] = xt

            nc.sync.dma_start(out=wq_sb[:], in_=wq8[:])
            nc.sync.dma_start(out=bq_sb[:], in_=bqT[:])
            issue_x_sp("q", 0)
            nc.sync.dma_start(out=wk_sb[:], in_=wk8[:])
            nc.sync.dma_start(out=bk_sb[:], in_=bkT[:])
            issue_x_sp("k", 0)
            issue_x_sp("q", 1)
            issue_x_sp("k", 1)
            emit_q(0, "q")
            emit_q(0, "k")
            emit_q(1, "q")
            nc.sync.dma_start(out=wv_sb[:], in_=wv8[:])
            nc.sync.dma_start(out=bvw_sb[:], in_=bvw[:])
            nc.sync.dma_start(out=ones_sb[:], in_=ones_c[:])

            mask_tiles = {}

            def mask_dma(cq, sk):
                mt = bm.tile([P, CQ], BF16, tag="m", name=f"m{cq}_{sk}")
                nc.sync.dma_start(
                    out=mt[:], in_=maskT[:, sk, cq * CQ : (cq + 1) * CQ]
                )
                mask_tiles[sk] = mt

            xatt_tiles = {}
            xaT_tiles = {}

            for cq in range(NCQ):
                xatt_t = bxa.tile([P, QS, E], BF16, tag="xatt")
                xatt_tiles[cq] = xatt_t
                mask_dma(cq, 0)
                mask_dma(cq, 1)
                for h in range(H):
                    half, ke = h & 1, h >> 1
                    pdsl = slice(64 * half, 64 * half + 64)
                    pTh = bp.tile([P, SK, CQ], BF16, tag="pT", name=f"pT{cq}_{h}")
                    for sk in range(SK):
                        if cq == 0 and h == 0:
                            if sk % 4 == 0 and sk > 0:
                                emit_q(sk // 4, "k")
                            if sk == 1:
                                issue_x("k", 2)
                            elif sk == 2:
                                issue_x("k", 3)
                            elif sk in (5, 7, 9):
                                issue_x("v", (sk - 5) // 2)
                            elif sk == 13:
                                issue_x("v", 3)
                        if cq == 0 and h == 1 and sk == 0:
                            nc.sync.dma_start(out=id_sb[:], in_=ident[:])
                            nc.sync.dma_start(out=wo_sb[:], in_=wo[:])
                        if cq == 0 and h == 2 and sk in (0, 2):
                            issue_x("q", 2 + sk // 2)
                        if h == 0 and sk + 2 < SK:
                            mask_dma(cq, sk + 2)
                        ss = psS.tile([P, CQ], F32, tag="s")
                        halves = 2 if (cq == 0 and h == 0 and sk < 4) else 1
                        et_t = be.tile([P, CQ], BF16, tag="e")
                        for piece in range(halves):
                            psl_ = slice(piece * (CQ // halves), (piece + 1) * (CQ // halves))
                            for n2 in range(CQ // 512 // halves):
                                base = piece * (CQ // halves) + n2 * 512
                                nsl = slice(base, base + 512)
                                gsl = slice(cq * CQ + base, cq * CQ + base + 512)
                                nc.tensor.matmul(
                                    ss[:, nsl], kT_sb[pdsl, ke, sk * P : (sk + 1) * P],
                                    qT_sb[pdsl, ke, gsl], start=True, stop=True,
                                )
                            nc.scalar.activation(et_t[:, psl_], ss[:, psl_], EXP)
                            nc.vector.tensor_mul(
                                pTh[:, sk, psl_], et_t[:, psl_], mask_tiles[sk][:, psl_]
                            )
                        if cq == 0 and h == 0:
                            pull(200)
                        else:
                            pull(2400 if sk < 6 else 1200)
                    # post-head work
                    if cq == 0 and h == 0:
                        for c in range(NCS):
                            for st in range(CS // P):
                                pending.append(
                                    ("v", 1900, (lambda c=c, st=st: emit_v(c, st)))
                                )
                    if cq == 0 and h == 2:
                        pending.append(("proj", 3400, lambda: emit_q(2, "q")))
                        pending.append(("proj", 3400, lambda: emit_q(3, "q")))
                    if h < H - 1:
                        for qsub in range(QS):
                            pending.append(("pv", SK * DK1 + 500, mk_pv(pTh, h, qsub, xatt_t)))
                    else:
                        # stagger PV with transpose/out-proj so the per-qsub
                        # chains pipeline through the 2-slot psum pools
                        xaT_t = bxt.tile([P, KE, CQ], BF16, tag="xaT")
                        xaT_tiles[cq] = xaT_t
                        last = cq == NCQ - 1
                        tr_pool, tr_tag = (psS, "s") if last else (None, "o2")
                        for qsub in range(QS):
                            pending.append(("pv", SK * DK1 + 500, mk_pv(pTh, h, qsub, xatt_t)))
                            if qsub >= 1:
                                j = qsub - 1
                                pending.append(("tr", 2 * P + 400, mk_tr(xatt_t, xaT_t, j, tr_pool, tr_tag, last)))
                                pending.append(("op", 2 * D + 600, mk_op(xaT_t, cq, j, last)))
                        for j in (QS - 1,):
                            pending.append(("tr", 2 * P + 400, mk_tr(xatt_t, xaT_t, j, tr_pool, tr_tag, last)))
                            pending.append(("op", 2 * D + 600, mk_op(xaT_t, cq, j, last)))
                if cq == 0:
                    flush("proj")
            while pending:
                _, _, f = pending.popleft()
                f()

    _split_multiwait(nc, 1)
    return nc


# ---------------------------------------------------------------- host side

B, S_FULL, D_FULL, H_FULL = 2, 2048, 1024, 16
DK_FULL = D_FULL // H_FULL
N_CORES = 8
GROUPS = N_CORES // B   # head-groups per batch
EG = D_FULL // GROUPS   # e-columns per core

_NC_CACHE = {}


def _get_program():
    if "full" not in _NC_CACHE:
        _NC_CACHE["full"] = build_program(D=D_FULL, S=S_FULL, E=EG, DK=DK_FULL)
    return _NC_CACHE["full"]


def _f8(a):
    import ml_dtypes

    return a.astype(ml_dtypes.float8_e4m3fn)


def _bf(a):
    import ml_dtypes

    return np.ascontiguousarray(a, dtype=np.float32).astype(ml_dtypes.bfloat16)


def _prep_x(aT, ncols):
    """[1024, ncols] f32 -> [128, ncols//512, 2, 4, 2, 512] fp8 (hi, lo*4)
    with d = 256*kt + 128*u + p; chunk-major for big-descriptor DMA."""
    a = np.ascontiguousarray(aT, dtype=np.float32)
    hi = _f8(a)
    lo4 = _f8((a - hi.astype(np.float32)) * 4.0)
    v = np.stack([hi, lo4], axis=0)
    v = v.reshape(2, 4, 2, 128, ncols).transpose(3, 0, 1, 2, 4)
    v = v.reshape(128, 2, 4, 2, ncols // 512, 512).transpose(0, 4, 1, 2, 3, 5)
    return np.ascontiguousarray(v)


def _prep_w(aT, ncols):
    """[1024, ncols] f32 (pre-scaled by WS) -> [128, 3, 4, 2, ncols] fp8
    versions (wh, wh/4, wl)."""
    a = np.ascontiguousarray(aT, dtype=np.float32)
    wh = _f8(a)
    whf = wh.astype(np.float32)
    wh4 = _f8(whf / 4.0)
    wl = _f8(a - whf)
    v = np.stack([wh, wh4, wl], axis=0)
    v = v.reshape(3, 4, 2, 128, ncols).transpose(3, 0, 1, 2, 4)
    return np.ascontiguousarray(v)


LAST_RES = None


def kernel(query, key, value, softmask, Wq, bq, Wk, bk, Wv, bv, Wo, bo, _trace=False):
    global LAST_RES
    from concourse.bass_utils import run_bass_kernel_spmd

    nc = _get_program()
    scale = np.float32(1.0 / math.sqrt(DK_FULL))

    x_cache = {}
    for b in range(B):
        x_cache[b] = (
            _prep_x(np.asarray(query[b], np.float32).T, S_FULL),
            _prep_x(np.asarray(key[b], np.float32).T, S_FULL),
            _prep_x(np.asarray(value[b], np.float32).T, S_FULL),
            np.ascontiguousarray(
                _bf(np.asarray(softmask[b], np.float32).T + 1e-30)
                .reshape(S_FULL // 128, 128, S_FULL)
                .transpose(1, 0, 2)
            ),
        )

    ident = _bf(np.eye(128, dtype=np.float32))
    ones_c = _bf(np.ones((1, 128), np.float32))

    in_maps = []
    for c in range(N_CORES):
        b, g = c // GROUPS, c % GROUPS
        es = slice(g * EG, (g + 1) * EG)
        xq8, xk8, xv8, mT = x_cache[b]
        m = {
            "xq8": xq8, "xk8": xk8, "xv8": xv8, "maskT": mT,
            "wq8": _prep_w(Wq[es, :].T * (scale * WS), EG),
            "wk8": _prep_w(Wk[es, :].T * WS, EG),
            "wv8": _prep_w(Wv[es, :].T * WS, EG),
            "wo": np.ascontiguousarray(
                _bf(Wo[:, es].T).reshape(EG // 128, 128, D_FULL).transpose(1, 0, 2)
            ),
            "bqT": np.ascontiguousarray(
                (np.asarray(bq[es], np.float32) * scale).reshape(EG // 128, 128).T
            ),
            "bkT": np.ascontiguousarray(
                np.asarray(bk[es], np.float32).reshape(EG // 128, 128).T
            ),
            "bvw": _bf(np.asarray(bv[es], np.float32)[None, :] * WS),
            "ones_c": ones_c,
            "ident": ident,
        }
        in_maps.append(m)

    res = run_bass_kernel_spmd(
        nc, in_maps, core_ids=list(range(N_CORES)), trace=_trace
    )
    LAST_RES = res

    outp = np.zeros((B, S_FULL, D_FULL), dtype=np.float32)
    for c in range(N_CORES):
        b = c // GROUPS
        o = res.results[c]["out"].astype(np.float32)  # [128, 16, D]
        outp[b] += o.transpose(1, 0, 2).reshape(S_FULL, D_FULL)
    outp += np.asarray(bo, dtype=np.float32)[None, None, :]
    return outp


# revision 40
# speedup vs baseline: 1.2336x; 1.0166x over previous
"""Multi-head attention (nn_Attention1D) on 8 Trainium2 NeuronCores.

Full inputs in, full output out.  Sharding: batch (2) x head-groups (4 heads
per core, E=256 e-columns).  Per-core pipeline (ACT exp stream is the
critical resource; everything else hides under it):

  QKV projections: compensated fp8 DoubleRow matmuls (3 terms:
      xh@wh + (xl*4)@(wh/4) + (xh/4)@(wl*4), weights pre-scaled by 64 into
      e4m3's normal range, rescaled in the bias-add) -> bf16-level accuracy
      at 1/4 the PE cost of bf16.  q/k stored bf16 [dk, s]; v stored
      bf16 [s, (h, dk|1)] with a ones column (softmax denominator for free).
  scores:   scoresT[sk, q] = kT.T @ qT per (head, sk-tile), fp32 PSUM.
  softmax:  ACT exp -> bf16; DVE multiply by softmask tile -> pT (bf16).
  PV:       flipped orientation: stationary = pT tile [k,q], moving =
            v [k, 65] -> xa[q, 64|denom] accumulated over sk (2x fewer
            streamed columns than the [dk, q] orientation).
  norm:     DVE reciprocal of the denom column + per-partition scalar mul.
  out-proj: PE-transpose xatt [q,e] -> xattT [e,q] via identity matmuls,
            then out[q, d] = xattT.T @ wo, copies on GPSIMD, bf16 out.
  Host sums the 4 per-core partials per batch and adds bo.
"""

import math
from collections import deque

import numpy as np

import concourse.bass as bass
import concourse.mybir as mybir
import concourse.tile as tile

F32 = mybir.dt.float32
BF16 = mybir.dt.bfloat16
F8 = mybir.dt.float8e4
DR = mybir.MatmulPerfMode.DoubleRow
EXP = mybir.ActivationFunctionType.Exp
MULT = mybir.AluOpType.mult
ADD = mybir.AluOpType.add

P = 128
WS = 64.0  # weight pre-scale into e4m3 normal range


def _split_multiwait(nc, max_waits=1):
    """This walrus build only accepts one sync wait per instruction; hoist
    extra waits onto NoOps inserted just before."""
    for bb in nc.main_func.blocks:
        new_insts = []
        for ins in bb.instructions:
            if ins.sync_info and len(ins.sync_info.on_wait) > max_waits:
                waits = list(ins.sync_info.on_wait)
                ins.sync_info.on_wait = waits[:max_waits]
                for i, w in enumerate(waits[max_waits:]):
                    nop = mybir.InstNoOp(name=f"{ins.name}_ws{i}", ins=[], outs=[])
                    nop.engine = ins.engine
                    nop.sync_info = mybir.SyncInfo(on_wait=[w], on_update=[])
                    nc.register_instruction(nop)
                    new_insts.append(nop)
            new_insts.append(ins)
        bb.instructions = new_insts


def build_program(D=1024, S=2048, E=256, DK=64):
    H = E // DK          # 4 heads per core
    KE = E // P          # 2 e-tiles
    KT = D // 256        # 4 DoubleRow k-tiles (K=256 each)
    SK = S // P          # 16 sk-tiles
    CS = 512             # projection chunk (s columns)
    NCS = S // CS        # 4
    CQ = 1024            # attention q chunk
    NCQ = S // CQ        # 2
    QS = CQ // P         # 8 q-subtiles per chunk
    DK1 = DK + 1

    nc = bass.Bass()
    xq8 = nc.dram_tensor("xq8", [P, S // 512, 2, KT, 2, 512], F8, kind="ExternalInput")
    xk8 = nc.dram_tensor("xk8", [P, S // 512, 2, KT, 2, 512], F8, kind="ExternalInput")
    xv8 = nc.dram_tensor("xv8", [P, S // 512, 2, KT, 2, 512], F8, kind="ExternalInput")
    wq8 = nc.dram_tensor("wq8", [P, 3, KT, 2, E], F8, kind="ExternalInput")
    wk8 = nc.dram_tensor("wk8", [P, 3, KT, 2, E], F8, kind="ExternalInput")
    wv8 = nc.dram_tensor("wv8", [P, 3, KT, 2, E], F8, kind="ExternalInput")
    wo = nc.dram_tensor("wo", [P, KE, D], BF16, kind="ExternalInput")
    bqT = nc.dram_tensor("bqT", [P, KE], F32, kind="ExternalInput")
    bkT = nc.dram_tensor("bkT", [P, KE], F32, kind="ExternalInput")
    bvw = nc.dram_tensor("bvw", [1, E], BF16, kind="ExternalInput")
    ones_c = nc.dram_tensor("ones_c", [1, P], BF16, kind="ExternalInput")
    ident = nc.dram_tensor("ident", [P, P], BF16, kind="ExternalInput")
    maskT = nc.dram_tensor("maskT", [P, SK, S], BF16, kind="ExternalInput")
    out = nc.dram_tensor("out", [P, S // P, D], BF16, kind="ExternalOutput")

    with tile.TileContext(nc) as tc:
        with (
            tc.tile_pool(name="persist", bufs=1) as persist,
            tc.tile_pool(name="ax", bufs=4) as ax,
            tc.tile_pool(name="bm", bufs=16) as bm,
            tc.tile_pool(name="be", bufs=4) as be,
            tc.tile_pool(name="bp", bufs=2) as bp,
            tc.tile_pool(name="bxa", bufs=2) as bxa,
            tc.tile_pool(name="bxt", bufs=1) as bxt,
            tc.tile_pool(name="bo", bufs=4) as bo_,
            tc.tile_pool(name="brc", bufs=4) as brc,
            tc.tile_pool(name="psS", bufs=2, space="PSUM") as psS,
            tc.tile_pool(name="psV", bufs=2, space="PSUM") as psV,
            tc.tile_pool(name="psO", bufs=2, space="PSUM") as psO,
        ):
            qT_sb = persist.tile([P, KE, S], BF16)
            kT_sb = persist.tile([P, KE, S], BF16)
            v_sb = persist.tile([P, SK, H, DK1], BF16)
            wq_sb = persist.tile([P, 3, KT, 2, E], F8)
            wk_sb = persist.tile([P, 3, KT, 2, E], F8)
            wv_sb = persist.tile([P, 3, KT, 2, E], F8)
            wo_sb = persist.tile([P, KE, D], BF16)
            bq_sb = persist.tile([P, KE], F32)
            bk_sb = persist.tile([P, KE], F32)
            bvw_sb = persist.tile([1, E], BF16)
            ones_sb = persist.tile([1, P], BF16)
            id_sb = persist.tile([P, P], BF16)
            nc.gpsimd.memset(v_sb[:, :, :, DK:DK1], 1.0)

            TERMS = [(0, 0), (1, 1), (0, 2)]  # (x ver, w ver): xh@wh + xl4@wh4 + xh@wl

            # ---------------- emission helpers ----------------
            x_tiles = {}

            def issue_x(which, c):
                xd = {"q": xq8, "k": xk8, "v": xv8}[which]
                xt = ax.tile([P, 2, KT, 2, CS], F8, tag="x", name=f"x{which}{c}")
                nc.gpsimd.dma_start(out=xt[:], in_=xd[:, c])
                x_tiles[(which, c)] = xt

            def emit_q(c, which):
                w_sb, b_sb, t_sb = {
                    "q": (wq_sb, bq_sb, qT_sb),
                    "k": (wk_sb, bk_sb, kT_sb),
                }[which]
                ssl = slice(c * CS, (c + 1) * CS)
                xt = x_tiles.pop((which, c))
                for et in range(KE):
                    esl = slice(et * P, (et + 1) * P)
                    ps = psV.tile([P, CS], F32, tag="v")
                    n = 0
                    for xv, wv in TERMS:
                        for kt in range(KT):
                            nc.tensor.matmul(
                                ps[:], w_sb[:, wv, kt, :, esl], xt[:, xv, kt, :, :],
                                start=(n == 0), stop=(n == 3 * KT - 1),
                                perf_mode=DR,
                            )
                            n += 1
                    nc.vector.tensor_scalar(
                        out=t_sb[:, et, ssl], in0=ps[:],
                        scalar1=1.0 / WS, scalar2=b_sb[:, et : et + 1],
                        op0=MULT, op1=ADD,
                    )

            xv_tiles = {}

            def emit_v(c, st):
                xt = x_tiles[("v", c)]
                stg = c * (CS // P) + st
                psl = slice(st * P, (st + 1) * P)
                ps = psO.tile([P, E], F32, tag="o2")
                n = 0
                for xv, wv in TERMS:
                    for kt in range(KT):
                        nc.tensor.matmul(
                            ps[:], xt[:, xv, kt, :, psl], wv_sb[:, wv, kt, :, :],
                            start=(n == 0), stop=False, perf_mode=DR,
                        )
                        n += 1
                nc.tensor.matmul(ps[:], ones_sb[:], bvw_sb[:], start=False, stop=True)
                nc.vector.tensor_scalar(
                    out=v_sb[:, stg, :, 0:DK],
                    in0=ps[:].rearrange("p (h d) -> p h d", h=H),
                    scalar1=1.0 / WS, scalar2=None, op0=MULT,
                )

            def mk_pv(pTh, h, qsub, xatt_t):
                def f():
                    xa = psV.tile([P, DK1], F32, tag="v")
                    qsl = slice(qsub * P, (qsub + 1) * P)
                    for sk in range(SK):
                        nc.tensor.matmul(
                            xa[:], pTh[:, sk, qsl], v_sb[:, sk, h, :],
                            start=(sk == 0), stop=(sk == SK - 1),
                        )
                    rec = brc.tile([P, 1], F32, tag="rc")
                    nc.vector.reciprocal(rec[:], xa[:, DK:DK1])
                    nc.vector.tensor_scalar(
                        out=xatt_t[:, qsub, h * DK : (h + 1) * DK],
                        in0=xa[:, 0:DK], scalar1=rec[:], scalar2=None, op0=MULT,
                    )
                    return SK * DK1 + 500
                return f

            def mk_tr(xatt_t, xaT_t, qsub, pool=None, ptag="o2", split_act=False):
                def f():
                    for et in range(KE):
                        pt = (pool or psO).tile([P, P], BF16, tag=ptag, name="pt")
                        nc.tensor.transpose(
                            pt[:], xatt_t[:, qsub, et * P : (et + 1) * P], id_sb[:]
                        )
                        dst = xaT_t[:, et, qsub * P : (qsub + 1) * P]
                        if split_act and et == 0:
                            nc.scalar.copy(out=dst, in_=pt[:])
                        else:
                            nc.vector.tensor_copy(dst, pt[:])
                    return 2 * P + 400
                return f

            def mk_op(xaT_t, cq, qsub, use_act=False, dn1_psv=False):
                def f():
                    ot = bo_.tile([P, D], BF16, tag="o")
                    qsl = slice(qsub * P, (qsub + 1) * P)
                    for dn in range(D // 512):
                        if dn1_psv and dn == 1:
                            po = psV.tile([P, 512], F32, tag="v", name="po1")
                        else:
                            po = psO.tile([P, 512], F32, tag="o2")
                        dsl = slice(dn * 512, (dn + 1) * 512)
                        for et in range(KE):
                            nc.tensor.matmul(
                                po[:], xaT_t[:, et, qsl], wo_sb[:, et, dsl],
                                start=(et == 0), stop=(et == KE - 1),
                            )
                        if use_act and dn == 0:
                            nc.scalar.copy(out=ot[:, dsl], in_=po[:])
                        else:
                            nc.vector.tensor_copy(ot[:, dsl], po[:])
                    if use_act:
                        nc.sync.dma_start(out=out[:, cq * QS + qsub, :], in_=ot[:])
                    else:
                        nc.gpsimd.dma_start(out=out[:, cq * QS + qsub, :], in_=ot[:])
                    return 2 * D + 600
                return f

            # ---------------- schedule ----------------
            pending = deque()  # (tag, cost_estimate, closure)

            def pull(budget):
                while pending and budget > 0:
                    tag, cost, f = pending.popleft()
                    r = f()
                    budget -= cost if r is None else r

            def flush(tag_needed):
                while any(t == tag_needed for t, _, _ in pending):
                    t, cost, f = pending.popleft()
                    f()

            # PE warmup: ramp the p-state to full clock before the first
            # projection data lands (dummy matmuls on a zeroed tile)
            warm_sb = persist.tile([1, 512], BF16, name="warm_sb")
            nc.gpsimd.memset(warm_sb[:], 0.0)
            for wi in range(12):
                ps_w = psS.tile([P, 512], F32, tag="s", name="wps")
                nc.tensor.matmul(
                    ps_w[:], warm_sb[:, 0:P], warm_sb[:], start=True, stop=True
                )

            # head: minimal DMA chain to the first scores: wq,xq0 / wk,xk0 / xq1
            # (head x chunks on the SP queue — they carry no WAR waits; later
            # chunks go through the Pool queue whose waits don't block issue)
            def issue_x_sp(which, c, split=False):
                xd = {"q": xq8, "k": xk8, "v": xv8}[which]
                xt = ax.tile([P, 2, KT, 2, CS], F8, tag="x", name=f"x{which}{c}")
                if split:
                    nc.sync.dma_start(out=xt[:, 0], in_=xd[:, c, 0])
                    nc.sync.dma_start(out=xt[:, 1], in_=xd[:, c, 1])
                else:
                    nc.sync.dma_start(out=xt[:], in_=xd[:, c])
                x_tiles[(which, c)] = xt

            mask_tiles = {}

            def mask_dma(cq, sk):
                mt = bm.tile([P, CQ], BF16, tag="m", name=f"m{cq}_{sk}")
                nc.sync.dma_start(
                    out=mt[:], in_=maskT[:, sk, cq * CQ : (cq + 1) * CQ]
                )
                mask_tiles[sk] = mt

            nc.sync.dma_start(out=wq_sb[:], in_=wq8[:])
            nc.sync.dma_start(out=bq_sb[:], in_=bqT[:])
            issue_x_sp("q", 0)
            nc.sync.dma_start(out=wk_sb[:], in_=wk8[:])
            nc.sync.dma_start(out=bk_sb[:], in_=bkT[:])
            issue_x_sp("k", 0)
            mask_dma(0, 0)
            mask_dma(0, 1)
            issue_x_sp("q", 1)
            issue_x_sp("k", 1)
            emit_q(0, "q")
            emit_q(0, "k")
            for wi in range(8):
                ps_w2 = psS.tile([P, 512], F32, tag="s", name="wps2")
                nc.tensor.matmul(
                    ps_w2[:], warm_sb[:, 0:P], warm_sb[:], start=True, stop=True
                )
            emit_q(1, "q")
            nc.sync.dma_start(out=wv_sb[:], in_=wv8[:])
            nc.sync.dma_start(out=bvw_sb[:], in_=bvw[:])
            nc.sync.dma_start(out=ones_sb[:], in_=ones_c[:])

            xatt_tiles = {}
            xaT_tiles = {}

            for cq in range(NCQ):
                xatt_t = bxa.tile([P, QS, E], BF16, tag="xatt")
                xatt_tiles[cq] = xatt_t
                if cq > 0:
                    mask_dma(cq, 0)
                    mask_dma(cq, 1)
                for h in range(H):
                    half, ke = h & 1, h >> 1
                    pdsl = slice(64 * half, 64 * half + 64)
                    pTh = bp.tile([P, SK, CQ], BF16, tag="pT", name=f"pT{cq}_{h}")
                    if h == 0:
                        # sk0/sk1 emitted piece-wise: the lower halves need
                        # only the first q chunk of this cq block, so the exp
                        # stream starts before the second chunk is projected
                        mask_dma(cq, 2)
                        mask_dma(cq, 3)
                        ss2 = [psS.tile([P, CQ], F32, tag="s", name=f"ss2_{i}")
                               for i in range(2)]
                        et2 = [be.tile([P, CQ], BF16, tag="e", name=f"et2_{i}")
                               for i in range(2)]
                        for piece in range(2):
                            if cq > 0 and piece == 1:
                                pull(4000)  # drain the deferred q-projection
                            for sk in range(2):
                                ss_, et_ = ss2[sk], et2[sk]
                                psl_ = slice(piece * 512, (piece + 1) * 512)
                                gsl_ = slice(cq * CQ + piece * 512,
                                             cq * CQ + (piece + 1) * 512)
                                nc.tensor.matmul(
                                    ss_[:, psl_],
                                    kT_sb[pdsl, ke, sk * P : (sk + 1) * P],
                                    qT_sb[pdsl, ke, gsl_],
                                    start=True, stop=True,
                                )
                                nc.scalar.activation(et_[:, psl_], ss_[:, psl_], EXP)
                                nc.vector.tensor_mul(
                                    pTh[:, sk, psl_], et_[:, psl_],
                                    mask_tiles[sk][:, psl_],
                                )
                        sk_range = range(2, SK)
                    else:
                        sk_range = range(SK)
                    for sk in sk_range:
                        if cq == 0 and h == 0:
                            if sk % 4 == 0 and sk > 0:
                                emit_q(sk // 4, "k")
                            if sk == 2:
                                issue_x("k", 2)
                            elif sk == 3:
                                issue_x("k", 3)
                            elif sk in (5, 7, 9):
                                issue_x("v", (sk - 5) // 2)
                            elif sk == 13:
                                issue_x("v", 3)
                        if cq == 0 and h == 1 and sk == 0:
                            nc.sync.dma_start(out=id_sb[:], in_=ident[:])
                            nc.sync.dma_start(out=wo_sb[:], in_=wo[:])
                        if cq == 0 and h == 2 and sk in (0, 2):
                            issue_x("q", 2 + sk // 2)
                        if h == 0 and sk + 2 < SK:
                            mask_dma(cq, sk + 2)
                        ss = psS.tile([P, CQ], F32, tag="s")
                        et_t = be.tile([P, CQ], BF16, tag="e")
                        for n2 in range(CQ // 512):
                            nsl = slice(n2 * 512, (n2 + 1) * 512)
                            gsl = slice(cq * CQ + n2 * 512, cq * CQ + (n2 + 1) * 512)
                            nc.tensor.matmul(
                                ss[:, nsl], kT_sb[pdsl, ke, sk * P : (sk + 1) * P],
                                qT_sb[pdsl, ke, gsl], start=True, stop=True,
                            )
                        nc.scalar.activation(et_t[:], ss[:], EXP)
                        nc.vector.tensor_mul(pTh[:, sk, :], et_t[:], mask_tiles[sk][:])
                        if cq == 0 and h == 0:
                            pull(200)
                        else:
                            pull(2400 if sk < 6 else 1200)
                    # post-head work
                    if cq == 0 and h == 0:
                        for c in range(NCS):
                            for st in range(CS // P):
                                pending.append(
                                    ("v", 1900, (lambda c=c, st=st: emit_v(c, st)))
                                )
                    if cq == 0 and h == 2:
                        pending.append(("proj", 3400, lambda: emit_q(2, "q")))
                        pending.append(("proj2", 3400, lambda: emit_q(3, "q")))
                    if h < H - 1:
                        for qsub in range(QS):
                            pending.append(("pv", SK * DK1 + 500, mk_pv(pTh, h, qsub, xatt_t)))
                    else:
                        # stagger PV with transpose/out-proj so the per-qsub
                        # chains pipeline through the 2-slot psum pools
                        xaT_t = bxt.tile([P, KE, CQ], BF16, tag="xaT")
                        xaT_tiles[cq] = xaT_t
                        last = cq == NCQ - 1
                        tr_pool, tr_tag = (psS, "s") if last else (None, "o2")
                        lag = 2 if last else 1
                        for qsub in range(QS):
                            pending.append(("pv", SK * DK1 + 500, mk_pv(pTh, h, qsub, xatt_t)))
                            if qsub >= lag:
                                j = qsub - lag
                                pending.append(("tr", 2 * P + 400, mk_tr(xatt_t, xaT_t, j, tr_pool, tr_tag, last)))
                                pending.append(("op", 2 * D + 600, mk_op(xaT_t, cq, j, last, last)))
                        for j in range(QS - lag, QS):
                            pending.append(("tr", 2 * P + 400, mk_tr(xatt_t, xaT_t, j, tr_pool, tr_tag, last)))
                            pending.append(("op", 2 * D + 600, mk_op(xaT_t, cq, j, last, last)))
                if cq == 0:
                    flush("proj")
            while pending:
                _, _, f = pending.popleft()
                f()

    _split_multiwait(nc, 1)
    return nc


# ---------------------------------------------------------------- host side

B, S_FULL, D_FULL, H_FULL = 2, 2048, 1024, 16
DK_FULL = D_FULL // H_FULL
N_CORES = 8
GROUPS = N_CORES // B   # head-groups per batch
EG = D_FULL // GROUPS   # e-columns per core

_NC_CACHE = {}


def _get_program():
    if "full" not in _NC_CACHE:
        _NC_CACHE["full"] = build_program(D=D_FULL, S=S_FULL, E=EG, DK=DK_FULL)
    return _NC_CACHE["full"]


def _f8(a):
    import ml_dtypes

    return a.astype(ml_dtypes.float8_e4m3fn)


def _bf(a):
    import ml_dtypes

    return np.ascontiguousarray(a, dtype=np.float32).astype(ml_dtypes.bfloat16)


def _prep_x(aT, ncols):
    """[1024, ncols] f32 -> [128, ncols//512, 2, 4, 2, 512] fp8 (hi, lo*4)
    with d = 256*kt + 128*u + p; chunk-major for big-descriptor DMA."""
    a = np.ascontiguousarray(aT, dtype=np.float32)
    hi = _f8(a)
    lo4 = _f8((a - hi.astype(np.float32)) * 4.0)
    v = np.stack([hi, lo4], axis=0)
    v = v.reshape(2, 4, 2, 128, ncols).transpose(3, 0, 1, 2, 4)
    v = v.reshape(128, 2, 4, 2, ncols // 512, 512).transpose(0, 4, 1, 2, 3, 5)
    return np.ascontiguousarray(v)


def _prep_w(aT, ncols):
    """[1024, ncols] f32 (pre-scaled by WS) -> [128, 3, 4, 2, ncols] fp8
    versions (wh, wh/4, wl)."""
    a = np.ascontiguousarray(aT, dtype=np.float32)
    wh = _f8(a)
    whf = wh.astype(np.float32)
    wh4 = _f8(whf / 4.0)
    wl = _f8(a - whf)
    v = np.stack([wh, wh4, wl], axis=0)
    v = v.reshape(3, 4, 2, 128, ncols).transpose(3, 0, 1, 2, 4)
    return np.ascontiguousarray(v)


LAST_RES = None


def kernel(query, key, value, softmask, Wq, bq, Wk, bk, Wv, bv, Wo, bo, _trace=False):
    global LAST_RES
    from concourse.bass_utils import run_bass_kernel_spmd

    nc = _get_program()
    scale = np.float32(1.0 / math.sqrt(DK_FULL))

    x_cache = {}
    for b in range(B):
        x_cache[b] = (
            _prep_x(np.asarray(query[b], np.float32).T, S_FULL),
            _prep_x(np.asarray(key[b], np.float32).T, S_FULL),
            _prep_x(np.asarray(value[b], np.float32).T, S_FULL),
            np.ascontiguousarray(
                _bf(np.asarray(softmask[b], np.float32).T + 1e-30)
                .reshape(S_FULL // 128, 128, S_FULL)
                .transpose(1, 0, 2)
            ),
        )

    ident = _bf(np.eye(128, dtype=np.float32))
    ones_c = _bf(np.ones((1, 128), np.float32))

    in_maps = []
    for c in range(N_CORES):
        b, g = c // GROUPS, c % GROUPS
        es = slice(g * EG, (g + 1) * EG)
        xq8, xk8, xv8, mT = x_cache[b]
        m = {
            "xq8": xq8, "xk8": xk8, "xv8": xv8, "maskT": mT,
            "wq8": _prep_w(Wq[es, :].T * (scale * WS), EG),
            "wk8": _prep_w(Wk[es, :].T * WS, EG),
            "wv8": _prep_w(Wv[es, :].T * WS, EG),
            "wo": np.ascontiguousarray(
                _bf(Wo[:, es].T).reshape(EG // 128, 128, D_FULL).transpose(1, 0, 2)
            ),
            "bqT": np.ascontiguousarray(
                (np.asarray(bq[es], np.float32) * scale).reshape(EG // 128, 128).T
            ),
            "bkT": np.ascontiguousarray(
                np.asarray(bk[es], np.float32).reshape(EG // 128, 128).T
            ),
            "bvw": _bf(np.asarray(bv[es], np.float32)[None, :] * WS),
            "ones_c": ones_c,
            "ident": ident,
        }
        in_maps.append(m)

    res = run_bass_kernel_spmd(
        nc, in_maps, core_ids=list(range(N_CORES)), trace=_trace
    )
    LAST_RES = res

    outp = np.zeros((B, S_FULL, D_FULL), dtype=np.float32)
    for c in range(N_CORES):
        b = c // GROUPS
        o = res.results[c]["out"].astype(np.float32)  # [128, 16, D]
        outp[b] += o.transpose(1, 0, 2).reshape(S_FULL, D_FULL)
    outp += np.asarray(bo, dtype=np.float32)[None, None, :]
    return outp


# revision 44
# speedup vs baseline: 1.2397x; 1.0049x over previous
"""Multi-head attention (nn_Attention1D) on 8 Trainium2 NeuronCores.

Full inputs in, full output out.  Sharding: batch (2) x head-groups (4 heads
per core, E=256 e-columns).  Per-core pipeline (ACT exp stream is the
critical resource; everything else hides under it):

  QKV projections: compensated fp8 DoubleRow matmuls (3 terms:
      xh@wh + (xl*4)@(wh/4) + (xh/4)@(wl*4), weights pre-scaled by 64 into
      e4m3's normal range, rescaled in the bias-add) -> bf16-level accuracy
      at 1/4 the PE cost of bf16.  q/k stored bf16 [dk, s]; v stored
      bf16 [s, (h, dk|1)] with a ones column (softmax denominator for free).
  scores:   scoresT[sk, q] = kT.T @ qT per (head, sk-tile), fp32 PSUM.
  softmax:  ACT exp -> bf16; DVE multiply by softmask tile -> pT (bf16).
  PV:       flipped orientation: stationary = pT tile [k,q], moving =
            v [k, 65] -> xa[q, 64|denom] accumulated over sk (2x fewer
            streamed columns than the [dk, q] orientation).
  norm:     DVE reciprocal of the denom column + per-partition scalar mul.
  out-proj: PE-transpose xatt [q,e] -> xattT [e,q] via identity matmuls,
            then out[q, d] = xattT.T @ wo, copies on GPSIMD, bf16 out.
  Host sums the 4 per-core partials per batch and adds bo.
"""

import math
from collections import deque

import numpy as np

import concourse.bass as bass
import concourse.mybir as mybir
import concourse.tile as tile

F32 = mybir.dt.float32
BF16 = mybir.dt.bfloat16
F8 = mybir.dt.float8e4
DR = mybir.MatmulPerfMode.DoubleRow
EXP = mybir.ActivationFunctionType.Exp
MULT = mybir.AluOpType.mult
ADD = mybir.AluOpType.add

P = 128
WS = 64.0  # weight pre-scale into e4m3 normal range


def _split_multiwait(nc, max_waits=1):
    """This walrus build only accepts one sync wait per instruction; hoist
    extra waits onto NoOps inserted just before."""
    for bb in nc.main_func.blocks:
        new_insts = []
        for ins in bb.instructions:
            if ins.sync_info and len(ins.sync_info.on_wait) > max_waits:
                waits = list(ins.sync_info.on_wait)
                ins.sync_info.on_wait = waits[:max_waits]
                for i, w in enumerate(waits[max_waits:]):
                    nop = mybir.InstNoOp(name=f"{ins.name}_ws{i}", ins=[], outs=[])
                    nop.engine = ins.engine
                    nop.sync_info = mybir.SyncInfo(on_wait=[w], on_update=[])
                    nc.register_instruction(nop)
                    new_insts.append(nop)
            new_insts.append(ins)
        bb.instructions = new_insts


def build_program(D=1024, S=2048, E=256, DK=64):
    H = E // DK          # 4 heads per core
    KE = E // P          # 2 e-tiles
    KT = D // 256        # 4 DoubleRow k-tiles (K=256 each)
    SK = S // P          # 16 sk-tiles
    CS = 512             # projection chunk (s columns)
    NCS = S // CS        # 4
    CQ = 1024            # attention q chunk
    NCQ = S // CQ        # 2
    QS = CQ // P         # 8 q-subtiles per chunk
    DK1 = DK + 1

    nc = bass.Bass()
    xq8 = nc.dram_tensor("xq8", [P, S // 512, 2, KT, 2, 512], F8, kind="ExternalInput")
    xk8 = nc.dram_tensor("xk8", [P, S // 512, 2, KT, 2, 512], F8, kind="ExternalInput")
    xv8 = nc.dram_tensor("xv8", [P, S // 512, 2, KT, 2, 512], F8, kind="ExternalInput")
    wq8 = nc.dram_tensor("wq8", [P, 3, KT, 2, E], F8, kind="ExternalInput")
    wk8 = nc.dram_tensor("wk8", [P, 3, KT, 2, E], F8, kind="ExternalInput")
    wv8 = nc.dram_tensor("wv8", [P, 3, KT, 2, E], F8, kind="ExternalInput")
    wo = nc.dram_tensor("wo", [P, KE, D], BF16, kind="ExternalInput")
    bqT = nc.dram_tensor("bqT", [P, KE], F32, kind="ExternalInput")
    bkT = nc.dram_tensor("bkT", [P, KE], F32, kind="ExternalInput")
    bvw = nc.dram_tensor("bvw", [1, E], BF16, kind="ExternalInput")
    ones_c = nc.dram_tensor("ones_c", [1, P], BF16, kind="ExternalInput")
    ident = nc.dram_tensor("ident", [P, P], BF16, kind="ExternalInput")
    maskT = nc.dram_tensor("maskT", [P, SK, S], BF16, kind="ExternalInput")
    out = nc.dram_tensor("out", [P, S // P, D], BF16, kind="ExternalOutput")

    with tile.TileContext(nc) as tc:
        with (
            tc.tile_pool(name="persist", bufs=1) as persist,
            tc.tile_pool(name="ax", bufs=4) as ax,
            tc.tile_pool(name="bm", bufs=16) as bm,
            tc.tile_pool(name="be", bufs=5) as be,
            tc.tile_pool(name="bp", bufs=2) as bp,
            tc.tile_pool(name="bxa", bufs=2) as bxa,
            tc.tile_pool(name="bxt", bufs=1) as bxt,
            tc.tile_pool(name="bo", bufs=3) as bo_,
            tc.tile_pool(name="brc", bufs=4) as brc,
            tc.tile_pool(name="psS", bufs=2, space="PSUM") as psS,
            tc.tile_pool(name="psV", bufs=2, space="PSUM") as psV,
            tc.tile_pool(name="psO", bufs=2, space="PSUM") as psO,
        ):
            qT_sb = persist.tile([P, KE, S], BF16)
            kT_sb = persist.tile([P, KE, S], BF16)
            v_sb = persist.tile([P, SK, H, DK1], BF16)
            wq_sb = persist.tile([P, 3, KT, 2, E], F8)
            wk_sb = persist.tile([P, 3, KT, 2, E], F8)
            wv_sb = persist.tile([P, 3, KT, 2, E], F8)
            wo_sb = persist.tile([P, KE, D], BF16)
            bq_sb = persist.tile([P, KE], F32)
            bk_sb = persist.tile([P, KE], F32)
            bvw_sb = persist.tile([1, E], BF16)
            ones_sb = persist.tile([1, P], BF16)
            id_sb = persist.tile([P, P], BF16)
            nc.gpsimd.memset(v_sb[:, :, :, DK:DK1], 1.0)

            TERMS = [(0, 0), (1, 1), (0, 2)]  # (x ver, w ver): xh@wh + xl4@wh4 + xh@wl

            # ---------------- emission helpers ----------------
            x_tiles = {}

            def issue_x(which, c):
                xd = {"q": xq8, "k": xk8, "v": xv8}[which]
                xt = ax.tile([P, 2, KT, 2, CS], F8, tag="x", name=f"x{which}{c}")
                nc.gpsimd.dma_start(out=xt[:], in_=xd[:, c])
                x_tiles[(which, c)] = xt

            def emit_q(c, which):
                w_sb, b_sb, t_sb = {
                    "q": (wq_sb, bq_sb, qT_sb),
                    "k": (wk_sb, bk_sb, kT_sb),
                }[which]
                ssl = slice(c * CS, (c + 1) * CS)
                xt = x_tiles.pop((which, c))
                for et in range(KE):
                    esl = slice(et * P, (et + 1) * P)
                    ps = psV.tile([P, CS], F32, tag="v")
                    n = 0
                    for xv, wv in TERMS:
                        for kt in range(KT):
                            nc.tensor.matmul(
                                ps[:], w_sb[:, wv, kt, :, esl], xt[:, xv, kt, :, :],
                                start=(n == 0), stop=(n == 3 * KT - 1),
                                perf_mode=DR,
                            )
                            n += 1
                    nc.vector.tensor_scalar(
                        out=t_sb[:, et, ssl], in0=ps[:],
                        scalar1=1.0 / WS, scalar2=b_sb[:, et : et + 1],
                        op0=MULT, op1=ADD,
                    )

            xv_tiles = {}

            def emit_v(c, st):
                xt = x_tiles[("v", c)]
                stg = c * (CS // P) + st
                psl = slice(st * P, (st + 1) * P)
                ps = psO.tile([P, E], F32, tag="o2")
                n = 0
                for xv, wv in TERMS:
                    for kt in range(KT):
                        nc.tensor.matmul(
                            ps[:], xt[:, xv, kt, :, psl], wv_sb[:, wv, kt, :, :],
                            start=(n == 0), stop=False, perf_mode=DR,
                        )
                        n += 1
                nc.tensor.matmul(ps[:], ones_sb[:], bvw_sb[:], start=False, stop=True)
                nc.vector.tensor_scalar(
                    out=v_sb[:, stg, :, 0:DK],
                    in0=ps[:].rearrange("p (h d) -> p h d", h=H),
                    scalar1=1.0 / WS, scalar2=None, op0=MULT,
                )

            def mk_pv(pTh, h, qsub, xatt_t):
                def f():
                    xa = psV.tile([P, DK1], F32, tag="v")
                    qsl = slice(qsub * P, (qsub + 1) * P)
                    for sk in range(SK):
                        nc.tensor.matmul(
                            xa[:], pTh[:, sk, qsl], v_sb[:, sk, h, :],
                            start=(sk == 0), stop=(sk == SK - 1),
                        )
                    rec = brc.tile([P, 1], F32, tag="rc")
                    nc.vector.reciprocal(rec[:], xa[:, DK:DK1])
                    nc.vector.tensor_scalar(
                        out=xatt_t[:, qsub, h * DK : (h + 1) * DK],
                        in0=xa[:, 0:DK], scalar1=rec[:], scalar2=None, op0=MULT,
                    )
                    return SK * DK1 + 500
                return f

            def mk_tr(xatt_t, xaT_t, qsub, pool=None, ptag="o2", split_act=False):
                def f():
                    for et in range(KE):
                        pt = (pool or psO).tile([P, P], BF16, tag=ptag, name="pt")
                        nc.tensor.transpose(
                            pt[:], xatt_t[:, qsub, et * P : (et + 1) * P], id_sb[:]
                        )
                        dst = xaT_t[:, et, qsub * P : (qsub + 1) * P]
                        if split_act and et == 0:
                            nc.scalar.copy(out=dst, in_=pt[:])
                        else:
                            nc.vector.tensor_copy(dst, pt[:])
                    return 2 * P + 400
                return f

            def mk_op(xaT_t, cq, qsub, use_act=False, dn1_psv=False):
                def f():
                    ot = bo_.tile([P, D], BF16, tag="o")
                    qsl = slice(qsub * P, (qsub + 1) * P)
                    for dn in range(D // 512):
                        if dn1_psv and dn == 1:
                            po = psV.tile([P, 512], F32, tag="v", name="po1")
                        else:
                            po = psO.tile([P, 512], F32, tag="o2")
                        dsl = slice(dn * 512, (dn + 1) * 512)
                        for et in range(KE):
                            nc.tensor.matmul(
                                po[:], xaT_t[:, et, qsl], wo_sb[:, et, dsl],
                                start=(et == 0), stop=(et == KE - 1),
                            )
                        if use_act and dn == 0:
                            nc.scalar.copy(out=ot[:, dsl], in_=po[:])
                        else:
                            nc.vector.tensor_copy(ot[:, dsl], po[:])
                    if use_act:
                        nc.sync.dma_start(out=out[:, cq * QS + qsub, :], in_=ot[:])
                    else:
                        nc.gpsimd.dma_start(out=out[:, cq * QS + qsub, :], in_=ot[:])
                    return 2 * D + 600
                return f

            # ---------------- schedule ----------------
            pending = deque()  # (tag, cost_estimate, closure)

            def pull(budget):
                while pending and budget > 0:
                    tag, cost, f = pending.popleft()
                    r = f()
                    budget -= cost if r is None else r

            def flush(tag_needed):
                while any(t == tag_needed for t, _, _ in pending):
                    t, cost, f = pending.popleft()
                    f()

            # PE warmup: ramp the p-state to full clock before the first
            # projection data lands (dummy matmuls on a zeroed tile)
            warm_sb = persist.tile([1, 512], BF16, name="warm_sb")
            nc.gpsimd.memset(warm_sb[:], 0.0)
            for wi in range(12):
                ps_w = psS.tile([P, 512], F32, tag="s", name="wps")
                nc.tensor.matmul(
                    ps_w[:], warm_sb[:, 0:P], warm_sb[:], start=True, stop=True
                )

            # head: minimal DMA chain to the first scores: wq,xq0 / wk,xk0 / xq1
            # (head x chunks on the SP queue — they carry no WAR waits; later
            # chunks go through the Pool queue whose waits don't block issue)
            def issue_x_sp(which, c, split=False):
                xd = {"q": xq8, "k": xk8, "v": xv8}[which]
                xt = ax.tile([P, 2, KT, 2, CS], F8, tag="x", name=f"x{which}{c}")
                if split:
                    nc.sync.dma_start(out=xt[:, 0], in_=xd[:, c, 0])
                    nc.sync.dma_start(out=xt[:, 1], in_=xd[:, c, 1])
                else:
                    nc.sync.dma_start(out=xt[:], in_=xd[:, c])
                x_tiles[(which, c)] = xt

            mask_tiles = {}

            def mask_dma(cq, sk):
                mt = bm.tile([P, CQ], BF16, tag="m", name=f"m{cq}_{sk}")
                nc.sync.dma_start(
                    out=mt[:], in_=maskT[:, sk, cq * CQ : (cq + 1) * CQ]
                )
                mask_tiles[sk] = mt

            nc.sync.dma_start(out=wq_sb[:], in_=wq8[:])
            nc.sync.dma_start(out=bq_sb[:], in_=bqT[:])
            issue_x_sp("q", 0)
            nc.sync.dma_start(out=wk_sb[:], in_=wk8[:])
            nc.sync.dma_start(out=bk_sb[:], in_=bkT[:])
            issue_x_sp("k", 0)
            mask_dma(0, 0)
            mask_dma(0, 1)
            issue_x_sp("q", 1)
            issue_x_sp("k", 1)
            emit_q(0, "q")
            emit_q(0, "k")
            for wi in range(8):
                ps_w2 = psS.tile([P, 512], F32, tag="s", name="wps2")
                nc.tensor.matmul(
                    ps_w2[:], warm_sb[:, 0:P], warm_sb[:], start=True, stop=True
                )
            emit_q(1, "q")
            nc.sync.dma_start(out=wv_sb[:], in_=wv8[:])
            nc.sync.dma_start(out=bvw_sb[:], in_=bvw[:])
            nc.sync.dma_start(out=ones_sb[:], in_=ones_c[:])

            xatt_tiles = {}
            xaT_tiles = {}

            for cq in range(NCQ):
                xatt_t = bxa.tile([P, QS, E], BF16, tag="xatt")
                xatt_tiles[cq] = xatt_t
                if cq > 0:
                    mask_dma(cq, 0)
                    mask_dma(cq, 1)
                for h in range(H):
                    half, ke = h & 1, h >> 1
                    pdsl = slice(64 * half, 64 * half + 64)
                    pTh = bp.tile([P, SK, CQ], BF16, tag="pT", name=f"pT{cq}_{h}")
                    if h == 0:
                        # sk0/sk1 emitted piece-wise: the lower halves need
                        # only the first q chunk of this cq block, so the exp
                        # stream starts before the second chunk is projected
                        mask_dma(cq, 2)
                        mask_dma(cq, 3)
                        ss2 = [psS.tile([P, CQ], F32, tag="s", name=f"ss2_{i}")
                               for i in range(2)]
                        et2 = [be.tile([P, CQ], BF16, tag="e", name=f"et2_{i}")
                               for i in range(2)]
                        for piece in range(2):
                            if cq > 0 and piece == 1:
                                pull(4000)  # drain the deferred q-projection
                            for sk in range(2):
                                ss_, et_ = ss2[sk], et2[sk]
                                psl_ = slice(piece * 512, (piece + 1) * 512)
                                gsl_ = slice(cq * CQ + piece * 512,
                                             cq * CQ + (piece + 1) * 512)
                                nc.tensor.matmul(
                                    ss_[:, psl_],
                                    kT_sb[pdsl, ke, sk * P : (sk + 1) * P],
                                    qT_sb[pdsl, ke, gsl_],
                                    start=True, stop=True,
                                )
                                nc.scalar.activation(et_[:, psl_], ss_[:, psl_], EXP)
                                nc.vector.tensor_mul(
                                    pTh[:, sk, psl_], et_[:, psl_],
                                    mask_tiles[sk][:, psl_],
                                )
                        sk_range = range(2, SK)
                    else:
                        sk_range = range(SK)
                    for sk in sk_range:
                        if cq == 0 and h == 0:
                            if sk % 4 == 0 and sk > 0:
                                emit_q(sk // 4, "k")
                            if sk == 2:
                                issue_x("k", 2)
                            elif sk == 3:
                                issue_x("k", 3)
                            elif sk in (5, 7, 9):
                                issue_x("v", (sk - 5) // 2)
                            elif sk == 13:
                                issue_x("v", 3)
                        if cq == 0 and h == 1 and sk == 0:
                            nc.sync.dma_start(out=id_sb[:], in_=ident[:])
                            nc.sync.dma_start(out=wo_sb[:], in_=wo[:])
                        if cq == 0 and h == 2 and sk in (0, 2):
                            issue_x("q", 2 + sk // 2)
                        if h == 0 and sk + 2 < SK:
                            mask_dma(cq, sk + 2)
                        ss = psS.tile([P, CQ], F32, tag="s")
                        et_t = be.tile([P, CQ], BF16, tag="e")
                        for n2 in range(CQ // 512):
                            nsl = slice(n2 * 512, (n2 + 1) * 512)
                            gsl = slice(cq * CQ + n2 * 512, cq * CQ + (n2 + 1) * 512)
                            nc.tensor.matmul(
                                ss[:, nsl], kT_sb[pdsl, ke, sk * P : (sk + 1) * P],
                                qT_sb[pdsl, ke, gsl], start=True, stop=True,
                            )
                        nc.scalar.activation(et_t[:], ss[:], EXP)
                        nc.vector.tensor_mul(pTh[:, sk, :], et_t[:], mask_tiles[sk][:])
                        if cq == 0 and h == 0:
                            pull(500 if sk > 9 else 200)
                        else:
                            pull(2400 if sk < 6 else 1200)
                    # post-head work
                    if cq == 0 and h == 0:
                        for c in range(NCS):
                            for st in range(CS // P):
                                pending.append(
                                    ("v", 1900, (lambda c=c, st=st: emit_v(c, st)))
                                )
                    if cq == 0 and h == 2:
                        pending.append(("proj", 3400, lambda: emit_q(2, "q")))
                        pending.append(("proj2", 3400, lambda: emit_q(3, "q")))
                    if h < H - 1:
                        for qsub in range(QS):
                            pending.append(("pv", SK * DK1 + 500, mk_pv(pTh, h, qsub, xatt_t)))
                    else:
                        # stagger PV with transpose/out-proj so the per-qsub
                        # chains pipeline through the 2-slot psum pools
                        xaT_t = bxt.tile([P, KE, CQ], BF16, tag="xaT")
                        xaT_tiles[cq] = xaT_t
                        last = cq == NCQ - 1
                        tr_pool, tr_tag = (psS, "s") if last else (None, "o2")
                        lag = 2 if last else 1
                        for qsub in range(QS):
                            pending.append(("pv", SK * DK1 + 500, mk_pv(pTh, h, qsub, xatt_t)))
                            if qsub >= lag:
                                j = qsub - lag
                                pending.append(("tr", 2 * P + 400, mk_tr(xatt_t, xaT_t, j, tr_pool, tr_tag, last)))
                                pending.append(("op", 2 * D + 600, mk_op(xaT_t, cq, j, last, last)))
                        for j in range(QS - lag, QS):
                            pending.append(("tr", 2 * P + 400, mk_tr(xatt_t, xaT_t, j, tr_pool, tr_tag, last)))
                            pending.append(("op", 2 * D + 600, mk_op(xaT_t, cq, j, last, last)))
                if cq == 0:
                    flush("proj")
            while pending:
                _, _, f = pending.popleft()
                f()

    _split_multiwait(nc, 1)
    return nc


# ---------------------------------------------------------------- host side

B, S_FULL, D_FULL, H_FULL = 2, 2048, 1024, 16
DK_FULL = D_FULL // H_FULL
N_CORES = 8
GROUPS = N_CORES // B   # head-groups per batch
EG = D_FULL // GROUPS   # e-columns per core

_NC_CACHE = {}


def _get_program():
    if "full" not in _NC_CACHE:
        _NC_CACHE["full"] = build_program(D=D_FULL, S=S_FULL, E=EG, DK=DK_FULL)
    return _NC_CACHE["full"]


def _f8(a):
    import ml_dtypes

    return a.astype(ml_dtypes.float8_e4m3fn)


def _bf(a):
    import ml_dtypes

    return np.ascontiguousarray(a, dtype=np.float32).astype(ml_dtypes.bfloat16)


def _prep_x(aT, ncols):
    """[1024, ncols] f32 -> [128, ncols//512, 2, 4, 2, 512] fp8 (hi, lo*4)
    with d = 256*kt + 128*u + p; chunk-major for big-descriptor DMA."""
    a = np.ascontiguousarray(aT, dtype=np.float32)
    hi = _f8(a)
    lo4 = _f8((a - hi.astype(np.float32)) * 4.0)
    v = np.stack([hi, lo4], axis=0)
    v = v.reshape(2, 4, 2, 128, ncols).transpose(3, 0, 1, 2, 4)
    v = v.reshape(128, 2, 4, 2, ncols // 512, 512).transpose(0, 4, 1, 2, 3, 5)
    return np.ascontiguousarray(v)


def _prep_w(aT, ncols):
    """[1024, ncols] f32 (pre-scaled by WS) -> [128, 3, 4, 2, ncols] fp8
    versions (wh, wh/4, wl)."""
    a = np.ascontiguousarray(aT, dtype=np.float32)
    wh = _f8(a)
    whf = wh.astype(np.float32)
    wh4 = _f8(whf / 4.0)
    wl = _f8(a - whf)
    v = np.stack([wh, wh4, wl], axis=0)
    v = v.reshape(3, 4, 2, 128, ncols).transpose(3, 0, 1, 2, 4)
    return np.ascontiguousarray(v)


LAST_RES = None


def kernel(query, key, value, softmask, Wq, bq, Wk, bk, Wv, bv, Wo, bo, _trace=False):
    global LAST_RES
    from concourse.bass_utils import run_bass_kernel_spmd

    nc = _get_program()
    scale = np.float32(1.0 / math.sqrt(DK_FULL))

    x_cache = {}
    for b in range(B):
        x_cache[b] = (
            _prep_x(np.asarray(query[b], np.float32).T, S_FULL),
            _prep_x(np.asarray(key[b], np.float32).T, S_FULL),
            _prep_x(np.asarray(value[b], np.float32).T, S_FULL),
            np.ascontiguousarray(
                _bf(np.asarray(softmask[b], np.float32).T + 1e-30)
                .reshape(S_FULL // 128, 128, S_FULL)
                .transpose(1, 0, 2)
            ),
        )

    ident = _bf(np.eye(128, dtype=np.float32))
    ones_c = _bf(np.ones((1, 128), np.float32))

    in_maps = []
    for c in range(N_CORES):
        b, g = c // GROUPS, c % GROUPS
        es = slice(g * EG, (g + 1) * EG)
        xq8, xk8, xv8, mT = x_cache[b]
        m = {
            "xq8": xq8, "xk8": xk8, "xv8": xv8, "maskT": mT,
            "wq8": _prep_w(Wq[es, :].T * (scale * WS), EG),
            "wk8": _prep_w(Wk[es, :].T * WS, EG),
            "wv8": _prep_w(Wv[es, :].T * WS, EG),
            "wo": np.ascontiguousarray(
                _bf(Wo[:, es].T).reshape(EG // 128, 128, D_FULL).transpose(1, 0, 2)
            ),
            "bqT": np.ascontiguousarray(
                (np.asarray(bq[es], np.float32) * scale).reshape(EG // 128, 128).T
            ),
            "bkT": np.ascontiguousarray(
                np.asarray(bk[es], np.float32).reshape(EG // 128, 128).T
            ),
            "bvw": _bf(np.asarray(bv[es], np.float32)[None, :] * WS),
            "ones_c": ones_c,
            "ident": ident,
        }
        in_maps.append(m)

    res = run_bass_kernel_spmd(
        nc, in_maps, core_ids=list(range(N_CORES)), trace=_trace
    )
    LAST_RES = res

    outp = np.zeros((B, S_FULL, D_FULL), dtype=np.float32)
    for c in range(N_CORES):
        b = c // GROUPS
        o = res.results[c]["out"].astype(np.float32)  # [128, 16, D]
        outp[b] += o.transpose(1, 0, 2).reshape(S_FULL, D_FULL)
    outp += np.asarray(bo, dtype=np.float32)[None, None, :]
    return outp
